# revision 1
# baseline (speedup 1.0000x reference)
"""Channel-attention block (QKV 1x1 -> L2-normalized channel attention ->
depthwise 3x3 -> 1x1 proj) for fixed shapes B=8, C=192, H=W=128, HEADS=16.

Self-contained: takes full unsharded inputs, returns full output.
Computation is data-parallel over the batch (the 8 images are independent);
this fallback evaluates the whole pipeline with BLAS-backed numpy so the
harness always gets a correct full-shape float32 output.
"""

import numpy as np

B, C, H, W = 8, 192, 128, 128
HEADS = 16
HD = C // HEADS
EPS = 1e-12


def kernel(x, w_qkv, w_dw, w_proj, temperature):
    x = np.asarray(x, dtype=np.float32)
    w_qkv = np.asarray(w_qkv, dtype=np.float32)
    w_dw = np.asarray(w_dw, dtype=np.float32)
    w_proj = np.asarray(w_proj, dtype=np.float32)
    temperature = np.asarray(temperature, dtype=np.float32)

    b, c, h, w = x.shape
    hw = h * w
    xf = x.reshape(b, c, hw)

    # 1x1 conv qkv projection: [b, 3c, hw]
    qkv = np.matmul(w_qkv[None], xf)
    q, k, v = qkv[:, :c], qkv[:, c : 2 * c], qkv[:, 2 * c :]

    def to_heads(t):
        return t.reshape(b, HEADS, HD, hw)

    q, k, v = to_heads(q), to_heads(k), to_heads(v)

    # L2-normalize along spatial dim with eps clamp
    qn = np.maximum(np.sqrt((q * q).sum(-1, keepdims=True)), EPS * EPS)
    kn = np.maximum(np.sqrt((k * k).sum(-1, keepdims=True)), EPS * EPS)
    qn = np.maximum(qn, EPS)
    kn = np.maximum(kn, EPS)
    q = q / qn
    k = k / kn

    # channel attention: [b, heads, hd, hd]
    attn = np.matmul(q, k.transpose(0, 1, 3, 2)) * temperature[None]
    attn = attn - attn.max(axis=-1, keepdims=True)
    np.exp(attn, out=attn)
    attn = attn / attn.sum(axis=-1, keepdims=True)

    out = np.matmul(attn, v).reshape(b, c, h, w)

    # depthwise 3x3 conv, padding=1
    p = np.pad(out, ((0, 0), (0, 0), (1, 1), (1, 1)))
    wd = w_dw[:, 0]  # [C, 3, 3]
    dw = np.zeros_like(out)
    for di in range(3):
        for dj in range(3):
            dw += wd[None, :, di : di + 1, dj : dj + 1] * p[:, :, di : di + h, dj : dj + w]

    # 1x1 conv projection
    y = np.matmul(w_proj[None], dw.reshape(b, c, hw))
    return y.reshape(b, c, h, w).astype(np.float32)



# revision 22
# speedup vs baseline: 1.5567x; 1.5567x over previous
"""Channel-attention block (QKV 1x1 -> L2-normalized channel attention ->
depthwise 3x3 -> 1x1 proj) on 8 Trainium2 NeuronCores, data-parallel over
the batch (B=8, C=192, H=W=128, HEADS=16, HD=12).

Key algebraic restructuring: q and k are never materialized on-chip.
With G = x @ x^T ([C,C] Gram over spatial dim), the attention logits are
  logits = Wq G Wk^T   (scaled by 1/(||q_c|| ||k_d||) * temperature)
and the norms are the diagonals of Wq G Wq^T / Wk G Wk^T.  Only v = Wv x
is computed at full spatial width.  The depthwise 3x3 conv runs as
per-channel FMAs on the Vector engine (bf16 2x mode via two padded copies
of the attention output at both column parities) with the first rows
offloaded to the Tensor engine as diagonal-matrix matmuls.
"""

import numpy as np
import ml_dtypes

B, C, H, W = 8, 192, 128, 128
HW = H * W
HEADS = 16
HD = C // HEADS
EPS = 1e-12

NCH = 32          # number of 512-column chunks of HW
DWT = 16          # dw row-tiles (8 rows each)
PE_DW_TILES = 3   # dw tiles 0..PE_DW_TILES-1 computed on PE, rest on DVE
CH = [(0, 128), (128, 64)]   # channel chunks (offset, size)

_CACHE = {}


def _build_bass():
    import concourse.bass as bass
    import concourse.mybir as mybir
    import concourse.tile as tile
    from concourse.masks import make_identity

    f32 = mybir.dt.float32
    bf16 = mybir.dt.bfloat16

    nc = bass.Bass()

    xh = nc.declare_dram_parameter("xh", [C, HW], bf16, isOutput=False)
    wvT = nc.declare_dram_parameter("wvT", [C, C], bf16, isOutput=False)
    wqT = nc.declare_dram_parameter("wqT", [C, C], f32, isOutput=False)
    wkT = nc.declare_dram_parameter("wkT", [C, C], f32, isOutput=False)
    wpT = nc.declare_dram_parameter("wpT", [C, C], bf16, isOutput=False)
    maskd = nc.declare_dram_parameter("maskd", [C, C], f32, isOutput=False)
    identd = nc.declare_dram_parameter("identd", [C, C], f32, isOutput=False)
    tempx = nc.declare_dram_parameter("tempx", [C, 1], f32, isOutput=False)
    dww = nc.declare_dram_parameter("dww", [C, 9], f32, isOutput=False)
    dwd1 = nc.declare_dram_parameter("dwd1", [9, 128, 128], bf16, isOutput=False)
    dwd2 = nc.declare_dram_parameter("dwd2", [9, 64, 64], bf16, isOutput=False)
    y = nc.declare_dram_parameter("y", [C, HW], f32, isOutput=True)

    with tile.TileContext(nc) as tc:
        _emit(tc, nc, bass, mybir, make_identity, f32, bf16,
              xh, wvT, wqT, wkT, wpT, maskd, identd, tempx, dww, dwd1, dwd2, y)

    patched = _spill_excess_waits(nc.to_json_bytes())
    nc.to_json_bytes = lambda: patched
    return nc


def _spill_excess_waits(bir_json: bytes) -> bytes:
    """walrus allows ~1 sync-wait per lowered ISA struct; Tile can attach
    several to one instruction.  Move every wait beyond the first onto an
    injected same-engine NoOp placed immediately before the instruction."""
    import json as _json

    j = _json.loads(bir_json)
    n = [0]
    for fn in j["functions"]:
        for blk in fn["blocks"]:
            out = []
            for inst in blk["instructions"]:
                si = inst.get("sync_info")
                keep = 0 if inst.get("opcode") == "ISA" else 1
                if (si and si.get("on_wait") and len(si["on_wait"]) > keep
                        and inst.get("opcode") != "EventSemaphore"):
                    waits = si["on_wait"]
                    for w in (waits[:-1] if keep else waits):
                        n[0] += 1
                        out.append({
                            "debug": inst.get("debug", 0),
                            "engine": inst["engine"],
                            "ins": [], "outs": [],
                            "name": f"WSPILL-{n[0]}",
                            "opcode": "NoOp",
                            "sync_info": {"on_update": [], "on_wait": [w]},
                        })
                    si["on_wait"] = [waits[-1]] if keep else []
                out.append(inst)
            blk["instructions"] = out
    return _json.dumps(j).encode()


def _emit(tc, nc, bass, mybir, make_identity, f32, bf16,
          xh, wvT, wqT, wkT, wpT, maskd, identd, tempx, dww, dwd1, dwd2, y):
    from contextlib import ExitStack

    Alu = mybir.AluOpType
    Act = mybir.ActivationFunctionType

    ctx = ExitStack()
    with ctx:
        konst = ctx.enter_context(tc.tile_pool(name="konst", bufs=1))
        st = ctx.enter_context(tc.tile_pool(name="st", bufs=2))
        drp = ctx.enter_context(tc.tile_pool(name="drp", bufs=1, space="DRAM"))

        # ---------- constant loads ----------
        def load_pair(src, dt, name):
            ts = []
            for ci, (off, sz) in enumerate(CH):
                t = konst.tile([sz, C], dt, tag=f"{name}{ci}")
                nc.gpsimd.dma_start(out=t, in_=src[off:off + sz, :])
                ts.append(t)
            return ts

        wvT_sb = load_pair(wvT, bf16, "wvT")
        wqT_sb = load_pair(wqT, f32, "wqT")
        wkT_sb = load_pair(wkT, f32, "wkT")
        wpT_sb = load_pair(wpT, bf16, "wpT")
        mask_sb = load_pair(maskd, f32, "mask")
        ident_sb = load_pair(identd, f32, "ident")

        temp_sb, dww_sb = [], []
        for ci, (off, sz) in enumerate(CH):
            t = konst.tile([sz, 1], f32, tag=f"temp{ci}")
            nc.gpsimd.dma_start(out=t, in_=tempx[off:off + sz, :])
            temp_sb.append(t)
            d = konst.tile([sz, 9], f32, tag=f"dww{ci}")
            nc.gpsimd.dma_start(out=d, in_=dww[off:off + sz, :])
            dww_sb.append(d)

        dwd_sb = []
        for ci, (off, sz) in enumerate(CH):
            t = konst.tile([sz, 9, sz], bf16, tag=f"dwd{ci}")
            src = (dwd1 if ci == 0 else dwd2).rearrange("s p m -> p s m")
            nc.gpsimd.dma_start(out=t, in_=src)
            dwd_sb.append(t)

        id128 = konst.tile([128, 128], f32, tag="id128")
        make_identity(nc, id128)
        zcol = konst.tile([128, 16, 1], bf16, tag="zcol")
        nc.gpsimd.memset(zcol, 0.0)
        zrow = konst.tile([128, 1, 132], bf16, tag="zrow")
        nc.gpsimd.memset(zrow, 0.0)

        # ---------- phase 1: G = x x^T via transposed loads ----------
        smctx = ExitStack()
        smp = smctx.enter_context(tc.tile_pool(name="smp", bufs=2, space="PSUM"))
        G_sb = []
        with tc.tile_pool(name="gx", bufs=1) as gxp:
            # one whole-tensor xbar-transpose load; the resulting spatial
            # grouping (stride-128 subsets per partition) is irrelevant for G.
            xT = gxp.tile([128, 128, C], bf16, tag="xT")
            nc.sync.dma_start(out=xT[:], in_=xh[:, :], transpose=True)
            for ci, (off, sz) in enumerate(CH):
                ps = smp.tile([sz, C], f32, tag="smps", bufs=3)
                for k in range(128):
                    nc.tensor.matmul(ps, lhsT=xT[:, k, off:off + sz],
                                     rhs=xT[:, k, :],
                                     start=(k == 0), stop=(k == 127))
                g = konst.tile([sz, C], f32, tag=f"G{ci}")
                nc.scalar.activation(out=g, in_=ps, func=Act.Copy)
                G_sb.append(g)

        big = ctx.enter_context(tc.tile_pool(name="big", bufs=1))

        # ---------- phase 2: attention smalls ----------
        def mm_small(lhsT_tiles, rhs_tiles, mslices, tag, dt=f32):
            """out[m, n] accumulated over the 2 K-chunks; returns psum tiles."""
            outs = []
            for mi, (moff, msz) in enumerate(mslices):
                ps = smp.tile([msz, C], f32, tag="smps", bufs=3)
                for kc in range(2):
                    nc.tensor.matmul(
                        ps, lhsT=lhsT_tiles[kc][:, moff:moff + msz],
                        rhs=rhs_tiles[kc], start=(kc == 0), stop=(kc == 1))
                outs.append(ps)
            return outs

        def to_sbuf(ps_tiles, tag, dt=f32):
            outs = []
            for ci, ps in enumerate(ps_tiles):
                t = st.tile([ps.shape[0], C], dt, tag=f"{tag}{ci}", bufs=1)
                nc.scalar.activation(out=t, in_=ps, func=Act.Copy)
                outs.append(t)
            return outs

        # T_c = G @ Wq^T ; T_b = G @ Wk^T   (lhsT = G, symmetric)
        Tc = to_sbuf(mm_small(G_sb, wqT_sb, CH, "tc"), "Tc")
        Tb = to_sbuf(mm_small(G_sb, wkT_sb, CH, "tb"), "Tb")

        # gram[c,d] = Wq G Wk^T : lhsT = Tc (=G Wq^T, so Tc^T rows=j), rhs=wkT
        gram_sb = to_sbuf(mm_small(Tc, wkT_sb, CH, "gram"), "gram")
        # qq[c,c'] = Wq G Wq^T : lhsT = wqT, rhs = Tc
        qq_ps = mm_small(wqT_sb, Tc, CH, "qq")
        kk_ps = mm_small(wkT_sb, Tb, CH, "kk")

        # ssq via masked row-reduce against identity
        ssq_q, ssq_k = [], []
        for ci, (off, sz) in enumerate(CH):
            scr = st.tile([sz, C], f32, tag=f"scr{ci}", bufs=1)
            scr2 = st.tile([sz, C], f32, tag=f"scr2{ci}", bufs=1)
            sq = st.tile([sz, 1], f32, tag=f"ssqq{ci}", bufs=1)
            nc.vector.tensor_mul(scr, qq_ps[ci], ident_sb[ci])
            nc.vector.reduce_sum(out=sq, in_=scr, axis=mybir.AxisListType.X)
            sk = st.tile([sz, 1], f32, tag=f"ssqk{ci}", bufs=1)
            nc.vector.tensor_mul(scr2, kk_ps[ci], ident_sb[ci])
            nc.vector.reduce_sum(out=sk, in_=scr2, axis=mybir.AxisListType.X)
            ssq_q.append(sq)
            ssq_k.append(sk)

        # scale_q[c] = temp[c] / max(sqrt(ssq_q), eps); rk = 1/max(sqrt(ssq_k),eps)
        scale_q, rk_col = [], []
        for ci, (off, sz) in enumerate(CH):
            a = st.tile([sz, 1], f32, tag=f"sq{ci}", bufs=1)
            nc.scalar.activation(out=a, in_=ssq_q[ci], func=Act.Sqrt)
            nc.vector.tensor_scalar_max(a, a, EPS)
            nc.vector.reciprocal(out=a, in_=a)
            nc.vector.tensor_mul(a, a, temp_sb[ci])
            scale_q.append(a)
            b = st.tile([sz, 1], f32, tag=f"rk{ci}", bufs=1)
            nc.scalar.activation(out=b, in_=ssq_k[ci], func=Act.Sqrt)
            nc.vector.tensor_scalar_max(b, b, EPS)
            nc.vector.reciprocal(out=b, in_=b)
            rk_col.append(b)

        # rk as a broadcast row: transpose [C,1] -> [1,C] on PE, bounce via DRAM
        rk_ps = smp.tile([1, C], f32, tag="smps", bufs=3)
        for ci, (off, sz) in enumerate(CH):
            nc.tensor.matmul(rk_ps[:, off:off + sz], lhsT=rk_col[ci],
                             rhs=id128[0:sz, 0:sz], is_transpose=True,
                             start=True, stop=True,
                             skip_group_check=True)
        rk_row = st.tile([1, C], f32, tag="rkrow", bufs=1)
        nc.scalar.activation(out=rk_row, in_=rk_ps, func=Act.Copy)
        rk_dram = drp.tile([1, C], f32, tag="rkdram")
        nc.sync.dma_start(out=rk_dram, in_=rk_row)
        rk_bc = []
        for ci, (off, sz) in enumerate(CH):
            t = st.tile([sz, C], f32, tag=f"rkbc{ci}", bufs=1)
            src = bass.AP(tensor=rk_dram.tensor, offset=rk_dram.offset,
                          ap=[[0, sz]] + list(rk_dram.ap[1:]))
            nc.gpsimd.dma_start(out=t, in_=src)
            rk_bc.append(t)

        # E = exp(scale_q[c] * rk[d] * gram[c,d]); mask; row-sum; recip
        rs = []
        Em = []
        for ci, (off, sz) in enumerate(CH):
            e = st.tile([sz, C], f32, tag=f"E{ci}", bufs=1)
            nc.vector.scalar_tensor_tensor(
                out=e, in0=gram_sb[ci], scalar=scale_q[ci], in1=rk_bc[ci],
                op0=Alu.mult, op1=Alu.mult)
            nc.scalar.activation(out=e, in_=e, func=Act.Exp)
            em = st.tile([sz, C], f32, tag=f"Em{ci}", bufs=1)
            srow = st.tile([sz, 1], f32, tag=f"srow{ci}", bufs=1)
            nc.vector.tensor_mul(em, e, mask_sb[ci])
            nc.vector.reduce_sum(out=srow, in_=em, axis=mybir.AxisListType.X)
            r = st.tile([sz, 1], f32, tag=f"rs{ci}", bufs=1)
            nc.vector.reciprocal(out=r, in_=srow)
            rs.append(r)
            Em.append(em)

        # A^T (block-diag softmax numerator, transposed) in bf16 for the matmul
        AT_sb = []
        for di, (doff, dsz) in enumerate(CH):
            ps = smp.tile([dsz, C], f32, tag="smps", bufs=3)
            for ci, (coff, csz) in enumerate(CH):
                nc.tensor.matmul(ps[:, coff:coff + csz],
                                 lhsT=Em[ci][:, doff:doff + dsz],
                                 rhs=id128[0:csz, 0:csz], is_transpose=True,
                                 start=True, stop=True,
                                 skip_group_check=True)
            at = st.tile([dsz, C], bf16, tag=f"AT{di}", bufs=1)
            nc.scalar.activation(out=at, in_=ps, func=Act.Copy)
            AT_sb.append(at)

        smctx.close()
        psp = ctx.enter_context(tc.tile_pool(name="psp", bufs=1, space="PSUM"))

        # ---------- phase 3: v = Wv x (full, resident, bf16) ----------
        v_sb = [big.tile([sz, HW], bf16, tag=f"v{ci}", name=f"v{ci}")
                for ci, (off, sz) in enumerate(CH)]
        for n in range(NCH):
            cols = slice(512 * n, 512 * (n + 1))
            xs = [st.tile([sz, 512], bf16, tag=f"xs{ci}", bufs=6, name=f"xs{ci}")
                  for ci, (off, sz) in enumerate(CH)]
            for ci, (off, sz) in enumerate(CH):
                nc.sync.dma_start(out=xs[ci], in_=xh[off:off + sz, cols])
            for mi, (moff, msz) in enumerate(CH):
                ps = psp.tile([msz, 512], f32, tag=f"mmps{mi}", bufs=2)
                for kc in range(2):
                    nc.tensor.matmul(ps, lhsT=wvT_sb[kc][:, moff:moff + msz],
                                     rhs=xs[kc], start=(kc == 0), stop=(kc == 1))
                nc.scalar.activation(out=v_sb[mi][:, cols], in_=ps, func=Act.Copy)

        # ---------- phase 4: u = (A v) / s, written padded at 2 parities ----
        # padded tiles: [sz, 10, 132]; A content at col 2 (serves dj=1 taps +
        # all PE-region taps), B content at col 3 (serves dj=0 and dj=2 taps).
        upA = [[big.tile([sz, 10, 132], bf16, tag=f"uA{ci}", bufs=4,
                         name=f"uA{ci}_{t}")
                for t in range(DWT)] for ci, (off, sz) in enumerate(CH)]
        upB = [[big.tile([sz, 10, 132], bf16, tag=f"uB{ci}", bufs=4,
                         name=f"uB{ci}_{t}")
                if t >= PE_DW_TILES else None
                for t in range(DWT)] for ci, (off, sz) in enumerate(CH)]

        def zc(dst, sz):    # zero a [sz, 10, 1] column strip via ACT
            nc.scalar.activation(out=dst, in_=zcol[0:sz, 0:10, :], func=Act.Copy)

        def zr(dst, sz):    # zero a [sz, 1, 132] row strip via ACT
            nc.scalar.activation(out=dst, in_=zrow[0:sz], func=Act.Copy)

        for ci, (off, sz) in enumerate(CH):
            for t in range(DWT):
                if upB[ci][t] is not None:
                    zc(upB[ci][t][:, :, 2:3], sz)
                    zc(upB[ci][t][:, :, 131:132], sz)
                if t < PE_DW_TILES:
                    # PE-region taps read A cols 1 (dj=0) and 130 (dj=2)
                    zc(upA[ci][t][:, :, 1:2], sz)
                    zc(upA[ci][t][:, :, 130:131], sz)
            zr(upA[ci][0][:, 0:1, :], sz)
            zr(upA[ci][DWT - 1][:, 9:10, :], sz)
            zr(upB[ci][DWT - 1][:, 9:10, :], sz)

        def u_write(mi, ps, rows_psum, t, lr, nrows):
            """copy psum rows [rows_psum, rows_psum+nrows) into tile t at
            local row lr (content rows are local 1..8)."""
            src = ps.rearrange("p (r w) -> p r w", w=128)[
                :, rows_psum:rows_psum + nrows, :]
            nc.scalar.activation(
                out=upA[mi][t][:, lr:lr + nrows, 2:130], in_=src,
                func=Act.Copy, scale=rs[mi])
            if upB[mi][t] is not None:
                nc.scalar.activation(
                    out=upB[mi][t][:, lr:lr + nrows, 3:131], in_=src,
                    func=Act.Copy, scale=rs[mi])

        for n in range(NCH):
            cols = slice(512 * n, 512 * (n + 1))
            t, half = n // 2, n % 2
            for mi, (moff, msz) in enumerate(CH):
                ps = psp.tile([msz, 512], f32, tag=f"mmps{mi}", bufs=2)
                for kc in range(2):
                    nc.tensor.matmul(ps, lhsT=AT_sb[kc][:, moff:moff + msz],
                                     rhs=v_sb[kc][:, cols],
                                     start=(kc == 0), stop=(kc == 1))
                u_write(mi, ps, 0, t, 1 + 4 * half, 4)
                if half == 0 and t >= 1:          # first row -> halo of t-1
                    u_write(mi, ps, 0, t - 1, 9, 1)
                if half == 1 and t <= DWT - 2:    # last row -> halo of t+1
                    u_write(mi, ps, 3, t + 1, 0, 1)

        # ---------- phase 5+6: depthwise 3x3 + projection, streamed --------
        # tap s = 3*di + dj reads local rows [di:di+8]; A cols [2:130] (dj=1),
        # B cols [2:130] (dj=0) / [4:132] (dj=2).
        def tap_src(ci, t, di, dj):
            if dj == 1:
                return upA[ci][t][:, di:di + 8, 2:130]
            return upB[ci][t][:, di:di + 8, 2 + dj:130 + dj]

        for t in range(DWT):
            dwo = [st.tile([sz, 8, 128], bf16, tag=f"dwo{ci}", bufs=3, name=f"dwo{ci}")
                   for ci, (off, sz) in enumerate(CH)]
            if t < PE_DW_TILES:
                for ci, (off, sz) in enumerate(CH):
                    for hf in range(2):
                        ps = psp.tile([sz, 512], f32, tag="dwps", bufs=1)
                        for s in range(9):
                            di, dj = s // 3, s % 3
                            src = upA[ci][t][:, di + 4 * hf:di + 4 * hf + 4,
                                             1 + dj:129 + dj]
                            nc.tensor.matmul(ps, lhsT=dwd_sb[ci][:, s, :],
                                             rhs=src, start=(s == 0),
                                             stop=(s == 8))
                        nc.scalar.activation(
                            out=dwo[ci][:, 4 * hf:4 * hf + 4, :], in_=ps,
                            func=Act.Copy)
            else:
                for ci, (off, sz) in enumerate(CH):
                    nc.vector.tensor_scalar_mul(
                        dwo[ci], tap_src(ci, t, 0, 0), dww_sb[ci][:, 0:1])
                    for s in range(1, 9):
                        di, dj = s // 3, s % 3
                        nc.vector.scalar_tensor_tensor(
                            out=dwo[ci], in0=tap_src(ci, t, di, dj),
                            scalar=dww_sb[ci][:, s:s + 1], in1=dwo[ci],
                            op0=Alu.mult, op1=Alu.add)

            for hf in range(2):
                cols = slice(1024 * t + 512 * hf, 1024 * t + 512 * (hf + 1))
                for mi, (moff, msz) in enumerate(CH):
                    ps = psp.tile([msz, 512], f32, tag="yps", bufs=2)
                    for kc in range(2):
                        nc.tensor.matmul(
                            ps, lhsT=wpT_sb[kc][:, moff:moff + msz],
                            rhs=dwo[kc][:, 4 * hf:4 * hf + 4, :],
                            start=(kc == 0), stop=(kc == 1))
                    ys = st.tile([msz, 512], f32, tag=f"ys{mi}", bufs=3)
                    nc.scalar.activation(out=ys, in_=ps, func=Act.Copy)
                    nc.sync.dma_start(out=y[moff:moff + msz, cols], in_=ys)


def _prep_host(x, w_qkv, w_dw, w_proj, temperature):
    bf = ml_dtypes.bfloat16
    wq, wk, wv = w_qkv[0:C], w_qkv[C:2 * C], w_qkv[2 * C:3 * C]
    base = {
        "wvT": np.ascontiguousarray(wv.T).astype(bf),
        "wqT": np.ascontiguousarray(wq.T).astype(np.float32),
        "wkT": np.ascontiguousarray(wk.T).astype(np.float32),
        "wpT": np.ascontiguousarray(w_proj.T).astype(bf),
        "tempx": np.repeat(temperature.reshape(HEADS), HD).reshape(C, 1)
                   .astype(np.float32),
        "dww": w_dw[:, 0].reshape(C, 9).astype(np.float32),
    }
    mask = np.zeros((C, C), np.float32)
    for h in range(HEADS):
        mask[h * HD:(h + 1) * HD, h * HD:(h + 1) * HD] = 1.0
    base["maskd"] = mask
    base["identd"] = np.eye(C, dtype=np.float32)
    d1 = np.zeros((9, 128, 128), np.float32)
    d2 = np.zeros((9, 64, 64), np.float32)
    wd = w_dw[:, 0].reshape(C, 9)
    for s in range(9):
        d1[s][np.arange(128), np.arange(128)] = wd[0:128, s]
        d2[s][np.arange(64), np.arange(64)] = wd[128:192, s]
    base["dwd1"] = d1.astype(bf)
    base["dwd2"] = d2.astype(bf)

    in_maps = []
    for i in range(B):
        m = dict(base)
        m["xh"] = np.ascontiguousarray(x[i].reshape(C, HW)).astype(bf)
        in_maps.append(m)
    return in_maps


def kernel(x, w_qkv, w_dw, w_proj, temperature):
    from concourse.bass_utils import run_bass_kernel_spmd

    x = np.asarray(x, np.float32)
    w_qkv = np.asarray(w_qkv, np.float32)
    w_dw = np.asarray(w_dw, np.float32)
    w_proj = np.asarray(w_proj, np.float32)
    temperature = np.asarray(temperature, np.float32)

    if "nc" not in _CACHE:
        _CACHE["nc"] = _build_bass()
    nc = _CACHE["nc"]

    in_maps = _prep_host(x, w_qkv, w_dw, w_proj, temperature)
    res = run_bass_kernel_spmd(nc, in_maps, core_ids=list(range(B)))
    _CACHE["last_results"] = res

    out = np.empty((B, C, H, W), np.float32)
    for i in range(B):
        out[i] = res.results[i]["y"].reshape(C, H, W)
    return out


# revision 27
# speedup vs baseline: 5.2998x; 3.4044x over previous
"""Channel-attention block (QKV 1x1 -> L2-normalized channel attention ->
depthwise 3x3 -> 1x1 proj) on 8 Trainium2 NeuronCores, data-parallel over
the batch (B=8, C=192, H=W=128, HEADS=16, HD=12).

Key algebraic restructuring: q and k are never materialized on-chip.
With G = x @ x^T ([C,C] Gram over spatial dim), the attention logits are
  logits = Wq G Wk^T   (scaled by 1/(||q_c|| ||k_d||) * temperature)
and the norms are the diagonals of Wq G Wq^T / Wk G Wk^T.  Only v = Wv x
is computed at full spatial width.  The depthwise 3x3 conv runs as
per-channel FMAs on the Vector engine (bf16 2x mode via two padded copies
of the attention output at both column parities) with the first rows
offloaded to the Tensor engine as diagonal-matrix matmuls.
"""

import numpy as np
import ml_dtypes

B, C, H, W = 8, 192, 128, 128
HW = H * W
HEADS = 16
HD = C // HEADS
EPS = 1e-12

NCH = 32          # number of 512-column chunks of HW
DWT = 16          # dw row-tiles (8 rows each)
PE_DW_TILES = 3   # dw tiles 0..PE_DW_TILES-1 computed on PE, rest on DVE
CH = [(0, 128), (128, 64)]   # channel chunks (offset, size)

_CACHE = {}


def _build_bass():
    import concourse.bass as bass
    import concourse.mybir as mybir
    import concourse.tile as tile
    from concourse.masks import make_identity

    f32 = mybir.dt.float32
    bf16 = mybir.dt.bfloat16

    nc = bass.Bass()

    xh = nc.declare_dram_parameter("xh", [C, HW], bf16, isOutput=False)
    wvT = nc.declare_dram_parameter("wvT", [C, C], bf16, isOutput=False)
    wqT = nc.declare_dram_parameter("wqT", [C, C], f32, isOutput=False)
    wkT = nc.declare_dram_parameter("wkT", [C, C], f32, isOutput=False)
    wpT = nc.declare_dram_parameter("wpT", [C, C], bf16, isOutput=False)
    maskd = nc.declare_dram_parameter("maskd", [C, C], f32, isOutput=False)
    identd = nc.declare_dram_parameter("identd", [C, C], f32, isOutput=False)
    tempx = nc.declare_dram_parameter("tempx", [C, 1], f32, isOutput=False)
    dww = nc.declare_dram_parameter("dww", [C, 9], f32, isOutput=False)
    dwd1 = nc.declare_dram_parameter("dwd1", [9, 128, 128], bf16, isOutput=False)
    dwd2 = nc.declare_dram_parameter("dwd2", [9, 64, 64], bf16, isOutput=False)
    y = nc.declare_dram_parameter("y", [C, HW], bf16, isOutput=True)

    with tile.TileContext(nc) as tc:
        _emit(tc, nc, bass, mybir, make_identity, f32, bf16,
              xh, wvT, wqT, wkT, wpT, maskd, identd, tempx, dww, dwd1, dwd2, y)

    patched = _spill_excess_waits(nc.to_json_bytes())
    nc.to_json_bytes = lambda: patched
    return nc


def _spill_excess_waits(bir_json: bytes) -> bytes:
    """walrus allows ~1 sync-wait per lowered ISA struct; Tile can attach
    several to one instruction.  Move every wait beyond the first onto an
    injected same-engine NoOp placed immediately before the instruction."""
    import json as _json

    j = _json.loads(bir_json)
    n = [0]
    for fn in j["functions"]:
        for blk in fn["blocks"]:
            out = []
            for inst in blk["instructions"]:
                si = inst.get("sync_info")
                keep = 0 if inst.get("opcode") == "ISA" else 1
                if (si and si.get("on_wait") and len(si["on_wait"]) > keep
                        and inst.get("opcode") != "EventSemaphore"):
                    waits = si["on_wait"]
                    for w in (waits[:-1] if keep else waits):
                        n[0] += 1
                        out.append({
                            "debug": inst.get("debug", 0),
                            "engine": inst["engine"],
                            "ins": [], "outs": [],
                            "name": f"WSPILL-{n[0]}",
                            "opcode": "NoOp",
                            "sync_info": {"on_update": [], "on_wait": [w]},
                        })
                    si["on_wait"] = [waits[-1]] if keep else []
                out.append(inst)
            blk["instructions"] = out
    return _json.dumps(j).encode()


def _emit(tc, nc, bass, mybir, make_identity, f32, bf16,
          xh, wvT, wqT, wkT, wpT, maskd, identd, tempx, dww, dwd1, dwd2, y):
    from contextlib import ExitStack

    Alu = mybir.AluOpType
    Act = mybir.ActivationFunctionType

    ctx = ExitStack()
    with ctx:
        konst = ctx.enter_context(tc.tile_pool(name="konst", bufs=1))
        st = ctx.enter_context(tc.tile_pool(name="st", bufs=2))
        drp = ctx.enter_context(tc.tile_pool(name="drp", bufs=1, space="DRAM"))

        # ---------- constant loads ----------
        def load_pair(src, dt, name):
            ts = []
            for ci, (off, sz) in enumerate(CH):
                t = konst.tile([sz, C], dt, tag=f"{name}{ci}")
                nc.gpsimd.dma_start(out=t, in_=src[off:off + sz, :])
                ts.append(t)
            return ts

        wvT_sb = load_pair(wvT, bf16, "wvT")
        wqT_sb = load_pair(wqT, f32, "wqT")
        wkT_sb = load_pair(wkT, f32, "wkT")
        wpT_sb = load_pair(wpT, bf16, "wpT")
        mask_sb = load_pair(maskd, f32, "mask")
        ident_sb = load_pair(identd, f32, "ident")

        temp_sb, dww_sb = [], []
        for ci, (off, sz) in enumerate(CH):
            t = konst.tile([sz, 1], f32, tag=f"temp{ci}")
            nc.gpsimd.dma_start(out=t, in_=tempx[off:off + sz, :])
            temp_sb.append(t)
            d = konst.tile([sz, 9], f32, tag=f"dww{ci}")
            nc.gpsimd.dma_start(out=d, in_=dww[off:off + sz, :])
            dww_sb.append(d)

        dwd_sb = []
        for ci, (off, sz) in enumerate(CH):
            t = konst.tile([sz, 9, sz], bf16, tag=f"dwd{ci}")
            src = (dwd1 if ci == 0 else dwd2).rearrange("s p m -> p s m")
            nc.gpsimd.dma_start(out=t, in_=src)
            dwd_sb.append(t)

        id128 = konst.tile([128, 128], f32, tag="id128")
        make_identity(nc, id128)
        zcol = konst.tile([128, 16, 1], bf16, tag="zcol")
        nc.gpsimd.memset(zcol, 0.0)
        zrow = konst.tile([128, 1, 132], bf16, tag="zrow")
        nc.gpsimd.memset(zrow, 0.0)

        # ---------- phase 1: G = x x^T via transposed loads ----------
        smctx = ExitStack()
        smp = smctx.enter_context(tc.tile_pool(name="smp", bufs=2, space="PSUM"))
        G_sb = []
        with tc.tile_pool(name="gx", bufs=1) as gxp:
            # one whole-tensor xbar-transpose load; the resulting spatial
            # grouping (stride-128 subsets per partition) is irrelevant for G.
            xT = gxp.tile([128, 128, C], bf16, tag="xT")
            nc.sync.dma_start(out=xT[:], in_=xh[:, :], transpose=True)
            for ci, (off, sz) in enumerate(CH):
                ps = smp.tile([sz, C], f32, tag="smps", bufs=3)
                for k in range(128):
                    nc.tensor.matmul(ps, lhsT=xT[:, k, off:off + sz],
                                     rhs=xT[:, k, :],
                                     start=(k == 0), stop=(k == 127))
                g = konst.tile([sz, C], f32, tag=f"G{ci}")
                nc.scalar.activation(out=g, in_=ps, func=Act.Copy)
                G_sb.append(g)

        big = ctx.enter_context(tc.tile_pool(name="big", bufs=1))

        # ---------- phase 2: attention smalls ----------
        def mm_small(lhsT_tiles, rhs_tiles, mslices, tag, dt=f32):
            """out[m, n] accumulated over the 2 K-chunks; returns psum tiles."""
            outs = []
            for mi, (moff, msz) in enumerate(mslices):
                ps = smp.tile([msz, C], f32, tag="smps", bufs=3)
                for kc in range(2):
                    nc.tensor.matmul(
                        ps, lhsT=lhsT_tiles[kc][:, moff:moff + msz],
                        rhs=rhs_tiles[kc], start=(kc == 0), stop=(kc == 1))
                outs.append(ps)
            return outs

        def to_sbuf(ps_tiles, tag, dt=f32):
            outs = []
            for ci, ps in enumerate(ps_tiles):
                t = st.tile([ps.shape[0], C], dt, tag=f"{tag}{ci}", bufs=1)
                nc.scalar.activation(out=t, in_=ps, func=Act.Copy)
                outs.append(t)
            return outs

        # T_c = G @ Wq^T ; T_b = G @ Wk^T   (lhsT = G, symmetric)
        Tc = to_sbuf(mm_small(G_sb, wqT_sb, CH, "tc"), "Tc")
        Tb = to_sbuf(mm_small(G_sb, wkT_sb, CH, "tb"), "Tb")

        # gram[c,d] = Wq G Wk^T : lhsT = Tc (=G Wq^T, so Tc^T rows=j), rhs=wkT
        gram_sb = to_sbuf(mm_small(Tc, wkT_sb, CH, "gram"), "gram")
        # qq[c,c'] = Wq G Wq^T : lhsT = wqT, rhs = Tc
        qq_ps = mm_small(wqT_sb, Tc, CH, "qq")
        kk_ps = mm_small(wkT_sb, Tb, CH, "kk")

        # ssq via masked row-reduce against identity
        ssq_q, ssq_k = [], []
        for ci, (off, sz) in enumerate(CH):
            scr = st.tile([sz, C], f32, tag=f"scr{ci}", bufs=1)
            scr2 = st.tile([sz, C], f32, tag=f"scr2{ci}", bufs=1)
            sq = st.tile([sz, 1], f32, tag=f"ssqq{ci}", bufs=1)
            nc.vector.tensor_mul(scr, qq_ps[ci], ident_sb[ci])
            nc.vector.reduce_sum(out=sq, in_=scr, axis=mybir.AxisListType.X)
            sk = st.tile([sz, 1], f32, tag=f"ssqk{ci}", bufs=1)
            nc.vector.tensor_mul(scr2, kk_ps[ci], ident_sb[ci])
            nc.vector.reduce_sum(out=sk, in_=scr2, axis=mybir.AxisListType.X)
            ssq_q.append(sq)
            ssq_k.append(sk)

        # scale_q[c] = temp[c] / max(sqrt(ssq_q), eps); rk = 1/max(sqrt(ssq_k),eps)
        scale_q, rk_col = [], []
        for ci, (off, sz) in enumerate(CH):
            a = st.tile([sz, 1], f32, tag=f"sq{ci}", bufs=1)
            nc.scalar.activation(out=a, in_=ssq_q[ci], func=Act.Sqrt)
            nc.vector.tensor_scalar_max(a, a, EPS)
            nc.vector.reciprocal(out=a, in_=a)
            nc.vector.tensor_mul(a, a, temp_sb[ci])
            scale_q.append(a)
            b = st.tile([sz, 1], f32, tag=f"rk{ci}", bufs=1)
            nc.scalar.activation(out=b, in_=ssq_k[ci], func=Act.Sqrt)
            nc.vector.tensor_scalar_max(b, b, EPS)
            nc.vector.reciprocal(out=b, in_=b)
            rk_col.append(b)

        # rk as a broadcast row: transpose [C,1] -> [1,C] on PE, bounce via DRAM
        rk_ps = smp.tile([1, C], f32, tag="smps", bufs=3)
        for ci, (off, sz) in enumerate(CH):
            nc.tensor.matmul(rk_ps[:, off:off + sz], lhsT=rk_col[ci],
                             rhs=id128[0:sz, 0:sz], is_transpose=True,
                             start=True, stop=True,
                             skip_group_check=True)
        rk_row = st.tile([1, C], f32, tag="rkrow", bufs=1)
        nc.scalar.activation(out=rk_row, in_=rk_ps, func=Act.Copy)
        rk_dram = drp.tile([1, C], f32, tag="rkdram")
        nc.sync.dma_start(out=rk_dram, in_=rk_row)
        rk_bc = []
        for ci, (off, sz) in enumerate(CH):
            t = st.tile([sz, C], f32, tag=f"rkbc{ci}", bufs=1)
            src = bass.AP(tensor=rk_dram.tensor, offset=rk_dram.offset,
                          ap=[[0, sz]] + list(rk_dram.ap[1:]))
            nc.gpsimd.dma_start(out=t, in_=src)
            rk_bc.append(t)

        # E = exp(scale_q[c] * rk[d] * gram[c,d]); mask; row-sum; recip
        rs = []
        Em = []
        for ci, (off, sz) in enumerate(CH):
            e = st.tile([sz, C], f32, tag=f"E{ci}", bufs=1)
            nc.vector.scalar_tensor_tensor(
                out=e, in0=gram_sb[ci], scalar=scale_q[ci], in1=rk_bc[ci],
                op0=Alu.mult, op1=Alu.mult)
            nc.scalar.activation(out=e, in_=e, func=Act.Exp)
            em = st.tile([sz, C], f32, tag=f"Em{ci}", bufs=1)
            srow = st.tile([sz, 1], f32, tag=f"srow{ci}", bufs=1)
            nc.vector.tensor_mul(em, e, mask_sb[ci])
            nc.vector.reduce_sum(out=srow, in_=em, axis=mybir.AxisListType.X)
            r = st.tile([sz, 1], f32, tag=f"rs{ci}", bufs=1)
            nc.vector.reciprocal(out=r, in_=srow)
            rs.append(r)
            Em.append(em)

        # A^T (block-diag softmax numerator, transposed) in bf16 for the matmul
        AT_sb = []
        for di, (doff, dsz) in enumerate(CH):
            ps = smp.tile([dsz, C], f32, tag="smps", bufs=3)
            for ci, (coff, csz) in enumerate(CH):
                nc.tensor.matmul(ps[:, coff:coff + csz],
                                 lhsT=Em[ci][:, doff:doff + dsz],
                                 rhs=id128[0:csz, 0:csz], is_transpose=True,
                                 start=True, stop=True,
                                 skip_group_check=True)
            at = st.tile([dsz, C], bf16, tag=f"AT{di}", bufs=1)
            nc.scalar.activation(out=at, in_=ps, func=Act.Copy)
            AT_sb.append(at)

        smctx.close()
        psp = ctx.enter_context(tc.tile_pool(name="psp", bufs=1, space="PSUM"))

        # ---------- phase 3: v = Wv x (full, resident, bf16) ----------
        v_sb = [big.tile([sz, HW], bf16, tag=f"v{ci}", name=f"v{ci}")
                for ci, (off, sz) in enumerate(CH)]
        for n in range(NCH):
            cols = slice(512 * n, 512 * (n + 1))
            xs = [st.tile([sz, 512], bf16, tag=f"xs{ci}", bufs=6, name=f"xs{ci}")
                  for ci, (off, sz) in enumerate(CH)]
            for ci, (off, sz) in enumerate(CH):
                nc.sync.dma_start(out=xs[ci], in_=xh[off:off + sz, cols])
            for mi, (moff, msz) in enumerate(CH):
                ps = psp.tile([msz, 512], f32, tag=f"mmps{mi}", bufs=2)
                for kc in range(2):
                    nc.tensor.matmul(ps, lhsT=wvT_sb[kc][:, moff:moff + msz],
                                     rhs=xs[kc], start=(kc == 0), stop=(kc == 1))
                nc.scalar.activation(out=v_sb[mi][:, cols], in_=ps, func=Act.Copy)

        # ---------- phase 4: u = (A v) / s, written padded at 2 parities ----
        # padded tiles: [sz, 10, 132]; A content at col 2 (serves dj=1 taps +
        # all PE-region taps), B content at col 3 (serves dj=0 and dj=2 taps).
        upA = [[big.tile([sz, 10, 132], bf16, tag=f"uA{ci}", bufs=4,
                         name=f"uA{ci}_{t}")
                for t in range(DWT)] for ci, (off, sz) in enumerate(CH)]
        upB = [[big.tile([sz, 10, 132], bf16, tag=f"uB{ci}", bufs=4,
                         name=f"uB{ci}_{t}")
                if t >= PE_DW_TILES else None
                for t in range(DWT)] for ci, (off, sz) in enumerate(CH)]

        def zc(dst, sz):    # zero a [sz, 10, 1] column strip via ACT
            nc.scalar.activation(out=dst, in_=zcol[0:sz, 0:10, :], func=Act.Copy)

        def zr(dst, sz):    # zero a [sz, 1, 132] row strip via ACT
            nc.scalar.activation(out=dst, in_=zrow[0:sz], func=Act.Copy)

        for ci, (off, sz) in enumerate(CH):
            for t in range(DWT):
                if upB[ci][t] is not None:
                    zc(upB[ci][t][:, :, 2:3], sz)
                    zc(upB[ci][t][:, :, 131:132], sz)
                if t < PE_DW_TILES:
                    # PE-region taps read A cols 1 (dj=0) and 130 (dj=2)
                    zc(upA[ci][t][:, :, 1:2], sz)
                    zc(upA[ci][t][:, :, 130:131], sz)
            zr(upA[ci][0][:, 0:1, :], sz)
            zr(upA[ci][DWT - 1][:, 9:10, :], sz)
            zr(upB[ci][DWT - 1][:, 9:10, :], sz)

        def u_write(mi, ps, rows_psum, t, lr, nrows):
            """copy psum rows [rows_psum, rows_psum+nrows) into tile t at
            local row lr (content rows are local 1..8)."""
            src = ps.rearrange("p (r w) -> p r w", w=128)[
                :, rows_psum:rows_psum + nrows, :]
            nc.scalar.activation(
                out=upA[mi][t][:, lr:lr + nrows, 2:130], in_=src,
                func=Act.Copy, scale=rs[mi])
            if upB[mi][t] is not None:
                nc.scalar.activation(
                    out=upB[mi][t][:, lr:lr + nrows, 3:131], in_=src,
                    func=Act.Copy, scale=rs[mi])

        for n in range(NCH):
            cols = slice(512 * n, 512 * (n + 1))
            t, half = n // 2, n % 2
            for mi, (moff, msz) in enumerate(CH):
                ps = psp.tile([msz, 512], f32, tag=f"mmps{mi}", bufs=2)
                for kc in range(2):
                    nc.tensor.matmul(ps, lhsT=AT_sb[kc][:, moff:moff + msz],
                                     rhs=v_sb[kc][:, cols],
                                     start=(kc == 0), stop=(kc == 1))
                u_write(mi, ps, 0, t, 1 + 4 * half, 4)
                if half == 0 and t >= 1:          # first row -> halo of t-1
                    u_write(mi, ps, 0, t - 1, 9, 1)
                if half == 1 and t <= DWT - 2:    # last row -> halo of t+1
                    u_write(mi, ps, 3, t + 1, 0, 1)

        # ---------- phase 5+6: depthwise 3x3 + projection, streamed --------
        # tap s = 3*di + dj reads local rows [di:di+8]; A cols [2:130] (dj=1),
        # B cols [2:130] (dj=0) / [4:132] (dj=2).
        def tap_src(ci, t, di, dj):
            if dj == 1:
                return upA[ci][t][:, di:di + 8, 2:130]
            return upB[ci][t][:, di:di + 8, 2 + dj:130 + dj]

        for t in range(DWT):
            dwo = [st.tile([sz, 8, 128], bf16, tag=f"dwo{ci}", bufs=3, name=f"dwo{ci}")
                   for ci, (off, sz) in enumerate(CH)]
            if t < PE_DW_TILES:
                for ci, (off, sz) in enumerate(CH):
                    for hf in range(2):
                        ps = psp.tile([sz, 512], f32, tag="dwps", bufs=1)
                        for s in range(9):
                            di, dj = s // 3, s % 3
                            src = upA[ci][t][:, di + 4 * hf:di + 4 * hf + 4,
                                             1 + dj:129 + dj]
                            nc.tensor.matmul(ps, lhsT=dwd_sb[ci][:, s, :],
                                             rhs=src, start=(s == 0),
                                             stop=(s == 8))
                        nc.scalar.activation(
                            out=dwo[ci][:, 4 * hf:4 * hf + 4, :], in_=ps,
                            func=Act.Copy)
            else:
                for ci, (off, sz) in enumerate(CH):
                    nc.vector.tensor_scalar_mul(
                        dwo[ci], tap_src(ci, t, 0, 0), dww_sb[ci][:, 0:1])
                    for s in range(1, 9):
                        di, dj = s // 3, s % 3
                        nc.vector.scalar_tensor_tensor(
                            out=dwo[ci], in0=tap_src(ci, t, di, dj),
                            scalar=dww_sb[ci][:, s:s + 1], in1=dwo[ci],
                            op0=Alu.mult, op1=Alu.add)

            for hf in range(2):
                cols = slice(1024 * t + 512 * hf, 1024 * t + 512 * (hf + 1))
                for mi, (moff, msz) in enumerate(CH):
                    ps = psp.tile([msz, 512], f32, tag="yps", bufs=2)
                    for kc in range(2):
                        nc.tensor.matmul(
                            ps, lhsT=wpT_sb[kc][:, moff:moff + msz],
                            rhs=dwo[kc][:, 4 * hf:4 * hf + 4, :],
                            start=(kc == 0), stop=(kc == 1))
                    ys = st.tile([msz, 512], bf16, tag=f"ys{mi}", bufs=3)
                    nc.scalar.activation(out=ys, in_=ps, func=Act.Copy)
                    nc.sync.dma_start(out=y[moff:moff + msz, cols], in_=ys)


def _prep_host(x, w_qkv, w_dw, w_proj, temperature):
    bf = ml_dtypes.bfloat16
    wq, wk, wv = w_qkv[0:C], w_qkv[C:2 * C], w_qkv[2 * C:3 * C]
    base = {
        "wvT": np.ascontiguousarray(wv.T).astype(bf),
        "wqT": np.ascontiguousarray(wq.T).astype(np.float32),
        "wkT": np.ascontiguousarray(wk.T).astype(np.float32),
        "wpT": np.ascontiguousarray(w_proj.T).astype(bf),
        "tempx": np.repeat(temperature.reshape(HEADS), HD).reshape(C, 1)
                   .astype(np.float32),
        "dww": w_dw[:, 0].reshape(C, 9).astype(np.float32),
    }
    mask = np.zeros((C, C), np.float32)
    for h in range(HEADS):
        mask[h * HD:(h + 1) * HD, h * HD:(h + 1) * HD] = 1.0
    base["maskd"] = mask
    base["identd"] = np.eye(C, dtype=np.float32)
    d1 = np.zeros((9, 128, 128), np.float32)
    d2 = np.zeros((9, 64, 64), np.float32)
    wd = w_dw[:, 0].reshape(C, 9)
    for s in range(9):
        d1[s][np.arange(128), np.arange(128)] = wd[0:128, s]
        d2[s][np.arange(64), np.arange(64)] = wd[128:192, s]
    base["dwd1"] = d1.astype(bf)
    base["dwd2"] = d2.astype(bf)

    in_maps = []
    for i in range(B):
        m = dict(base)
        m["xh"] = np.ascontiguousarray(x[i].reshape(C, HW)).astype(bf)
        in_maps.append(m)
    return in_maps


def _get_runner():
    """Build the jitted 8-core SPMD executor once and cache it; a fresh
    jax.jit per call would re-lower the whole module every time."""
    if "runner" in _CACHE:
        return _CACHE["runner"]
    import jax
    from jax.experimental.shard_map import shard_map
    from jax.sharding import Mesh, PartitionSpec
    import concourse.mybir as mybir
    from concourse import bass2jax

    nc = _CACHE.get("nc")
    if nc is None:
        nc = _CACHE["nc"] = _build_bass()
    bass2jax.install_neuronx_cc_hook()

    partition_name = (nc.partition_id_tensor.name
                      if nc.partition_id_tensor else None)
    in_names, out_names, out_avals, zero_shapes = [], [], [], []
    for alloc in nc.m.functions[0].allocations:
        if not isinstance(alloc, mybir.MemoryLocationSet):
            continue
        name = alloc.memorylocations[0].name
        if alloc.kind == "ExternalInput":
            if name != partition_name:
                in_names.append(name)
        elif alloc.kind == "ExternalOutput":
            shape = tuple(alloc.tensor_shape)
            dtype = mybir.dt.np(alloc.dtype)
            out_names.append(name)
            out_avals.append(jax.core.ShapedArray(shape, dtype))
            zero_shapes.append((shape, dtype))
    n_params = len(in_names)
    all_names = in_names + out_names
    if partition_name is not None:
        all_names = all_names + [partition_name]
    donate = tuple(range(n_params, n_params + len(out_names)))

    def _body(*args):
        operands = list(args)
        if partition_name is not None:
            operands.append(bass2jax.partition_id_tensor())
        outs = bass2jax._bass_exec_p.bind(
            *operands,
            out_avals=tuple(out_avals),
            in_names=tuple(all_names),
            out_names=tuple(out_names),
            lowering_input_output_aliases=(),
            sim_require_finite=True,
            sim_require_nnan=True,
            nc=nc,
        )
        return tuple(outs)

    devices = jax.devices()[:B]
    mesh = Mesh(np.asarray(devices), ("core",))
    specs = (PartitionSpec("core"),) * (n_params + len(out_names))
    fn = jax.jit(
        shard_map(_body, mesh=mesh, in_specs=specs,
                  out_specs=(PartitionSpec("core"),) * len(out_names),
                  check_rep=False),
        donate_argnums=donate, keep_unused=True)

    import jax.numpy as jnp
    from jax.sharding import NamedSharding

    def _mk_zeros():
        return tuple(jnp.zeros((B * s[0], *s[1:]), dt) for (s, dt) in zero_shapes)

    zfn = jax.jit(_mk_zeros,
                  out_shardings=tuple(NamedSharding(mesh, PartitionSpec("core"))
                                      for _ in zero_shapes))
    _CACHE["zeros_fn"] = zfn

    def _chain(iters):
        def f(*args):
            xs = args[0]
            outs = _body(*args)
            for _ in range(iters - 1):
                dep = (outs[0].reshape(-1)[0] * 0).astype(xs.dtype)
                zs = tuple(jnp.zeros((s[0], *s[1:]), dt)
                           for (s, dt) in zero_shapes)
                outs = _body(xs + dep, *args[1:n_params], *zs)
            return outs
        return jax.jit(
            shard_map(f, mesh=mesh, in_specs=specs,
                      out_specs=(PartitionSpec("core"),) * len(out_names),
                      check_rep=False), keep_unused=True)

    _CACHE["chain_fn"] = _chain
    _CACHE["runner"] = (fn, in_names, out_names, out_avals, zero_shapes, n_params)
    return _CACHE["runner"]


def measure_device_ns(in_maps=None, iters=9):
    """Per-run device-exec estimate: run the kernel `iters` times chained
    inside one jit (serialized by a data dependency) and take the slope.
    Transfers amortize away; the chain link adds ~one 6 MB elementwise
    pass per iter, so this slightly over-estimates."""
    import jax, time
    fn, in_names, out_names, out_avals, zero_shapes, n_params = _get_runner()
    if in_maps is None:
        in_maps = _CACHE["last_in_maps"]
    concat_in = [
        np.concatenate([in_maps[c][name] for c in range(B)], axis=0)
        for name in in_names
    ]

    def run(k):
        f = _CACHE["chain_fn"](k)
        zs = _CACHE["zeros_fn"]()
        out = f(*concat_in, *zs)
        jax.block_until_ready(out)        # warm/compile
        ts = []
        for _ in range(3):
            zs = _CACHE["zeros_fn"]()
            t0 = time.perf_counter()
            out = f(*concat_in, *zs)
            jax.block_until_ready(out)
            ts.append(time.perf_counter() - t0)
        return min(ts)

    t1, tk = run(1), run(iters)
    return (tk - t1) / (iters - 1) * 1e9


def kernel(x, w_qkv, w_dw, w_proj, temperature):
    x = np.asarray(x, np.float32)
    w_qkv = np.asarray(w_qkv, np.float32)
    w_dw = np.asarray(w_dw, np.float32)
    w_proj = np.asarray(w_proj, np.float32)
    temperature = np.asarray(temperature, np.float32)

    fn, in_names, out_names, out_avals, zero_shapes, n_params = _get_runner()
    in_maps = _prep_host(x, w_qkv, w_dw, w_proj, temperature)
    _CACHE["last_in_maps"] = in_maps

    concat_in = [
        np.concatenate([in_maps[c][name] for c in range(B)], axis=0)
        for name in in_names
    ]
    concat_zeros = _CACHE["zeros_fn"]()
    out_arrs = fn(*concat_in, *concat_zeros)
    y = np.asarray(out_arrs[0]).astype(np.float32).reshape(B, C, H, W)
    return y


# revision 28
# speedup vs baseline: 974.9781x; 183.9644x over previous
"""Channel-attention block (QKV 1x1 -> L2-normalized channel attention ->
depthwise 3x3 -> 1x1 proj) on 8 Trainium2 NeuronCores, data-parallel over
the batch (B=8, C=192, H=W=128, HEADS=16, HD=12).

Key algebraic restructuring: q and k are never materialized on-chip.
With G = x @ x^T ([C,C] Gram over spatial dim), the attention logits are
  logits = Wq G Wk^T   (scaled by 1/(||q_c|| ||k_d||) * temperature)
and the norms are the diagonals of Wq G Wq^T / Wk G Wk^T.  Only v = Wv x
is computed at full spatial width.  The depthwise 3x3 conv runs as
per-channel FMAs on the Vector engine (bf16 2x mode via two padded copies
of the attention output at both column parities) with the first rows
offloaded to the Tensor engine as diagonal-matrix matmuls.
"""

import numpy as np
import ml_dtypes

B, C, H, W = 8, 192, 128, 128
HW = H * W
HEADS = 16
HD = C // HEADS
EPS = 1e-12

NCH = 32          # number of 512-column chunks of HW
DWT = 16          # dw row-tiles (8 rows each)
PE_DW_TILES = 3   # dw tiles 0..PE_DW_TILES-1 computed on PE, rest on DVE
CH = [(0, 128), (128, 64)]   # channel chunks (offset, size)

_CACHE = {}


def _build_bass():
    import concourse.bass as bass
    import concourse.mybir as mybir
    import concourse.tile as tile
    from concourse.masks import make_identity

    f32 = mybir.dt.float32
    bf16 = mybir.dt.bfloat16

    nc = bass.Bass()

    xh = nc.declare_dram_parameter("xh", [C, HW], bf16, isOutput=False)
    wvT = nc.declare_dram_parameter("wvT", [C, C], bf16, isOutput=False)
    wqT = nc.declare_dram_parameter("wqT", [C, C], f32, isOutput=False)
    wkT = nc.declare_dram_parameter("wkT", [C, C], f32, isOutput=False)
    wpT = nc.declare_dram_parameter("wpT", [C, C], bf16, isOutput=False)
    maskd = nc.declare_dram_parameter("maskd", [C, C], f32, isOutput=False)
    identd = nc.declare_dram_parameter("identd", [C, C], f32, isOutput=False)
    tempx = nc.declare_dram_parameter("tempx", [C, 1], f32, isOutput=False)
    dww = nc.declare_dram_parameter("dww", [C, 9], f32, isOutput=False)
    dwd1 = nc.declare_dram_parameter("dwd1", [9, 128, 128], bf16, isOutput=False)
    dwd2 = nc.declare_dram_parameter("dwd2", [9, 64, 64], bf16, isOutput=False)
    y = nc.declare_dram_parameter("y", [C, HW], bf16, isOutput=True)

    with tile.TileContext(nc) as tc:
        _emit(tc, nc, bass, mybir, make_identity, f32, bf16,
              xh, wvT, wqT, wkT, wpT, maskd, identd, tempx, dww, dwd1, dwd2, y)

    patched = _spill_excess_waits(nc.to_json_bytes())
    nc.to_json_bytes = lambda: patched
    return nc


def _spill_excess_waits(bir_json: bytes) -> bytes:
    """walrus allows ~1 sync-wait per lowered ISA struct; Tile can attach
    several to one instruction.  Move every wait beyond the first onto an
    injected same-engine NoOp placed immediately before the instruction."""
    import json as _json

    j = _json.loads(bir_json)
    n = [0]
    for fn in j["functions"]:
        for blk in fn["blocks"]:
            out = []
            for inst in blk["instructions"]:
                si = inst.get("sync_info")
                keep = 0 if inst.get("opcode") == "ISA" else 1
                if (si and si.get("on_wait") and len(si["on_wait"]) > keep
                        and inst.get("opcode") != "EventSemaphore"):
                    waits = si["on_wait"]
                    for w in (waits[:-1] if keep else waits):
                        n[0] += 1
                        out.append({
                            "debug": inst.get("debug", 0),
                            "engine": inst["engine"],
                            "ins": [], "outs": [],
                            "name": f"WSPILL-{n[0]}",
                            "opcode": "NoOp",
                            "sync_info": {"on_update": [], "on_wait": [w]},
                        })
                    si["on_wait"] = [waits[-1]] if keep else []
                out.append(inst)
            blk["instructions"] = out
    return _json.dumps(j).encode()


def _emit(tc, nc, bass, mybir, make_identity, f32, bf16,
          xh, wvT, wqT, wkT, wpT, maskd, identd, tempx, dww, dwd1, dwd2, y):
    from contextlib import ExitStack

    Alu = mybir.AluOpType
    Act = mybir.ActivationFunctionType

    ctx = ExitStack()
    with ctx:
        konst = ctx.enter_context(tc.tile_pool(name="konst", bufs=1))
        st = ctx.enter_context(tc.tile_pool(name="st", bufs=2))
        drp = ctx.enter_context(tc.tile_pool(name="drp", bufs=1, space="DRAM"))

        # ---------- constant loads ----------
        def load_pair(src, dt, name):
            ts = []
            for ci, (off, sz) in enumerate(CH):
                t = konst.tile([sz, C], dt, tag=f"{name}{ci}")
                nc.gpsimd.dma_start(out=t, in_=src[off:off + sz, :])
                ts.append(t)
            return ts

        wvT_sb = load_pair(wvT, bf16, "wvT")
        wqT_sb = load_pair(wqT, f32, "wqT")
        wkT_sb = load_pair(wkT, f32, "wkT")
        wpT_sb = load_pair(wpT, bf16, "wpT")
        mask_sb = load_pair(maskd, f32, "mask")
        ident_sb = load_pair(identd, f32, "ident")

        temp_sb, dww_sb = [], []
        for ci, (off, sz) in enumerate(CH):
            t = konst.tile([sz, 1], f32, tag=f"temp{ci}")
            nc.gpsimd.dma_start(out=t, in_=tempx[off:off + sz, :])
            temp_sb.append(t)
            d = konst.tile([sz, 9], f32, tag=f"dww{ci}")
            nc.gpsimd.dma_start(out=d, in_=dww[off:off + sz, :])
            dww_sb.append(d)

        dwd_sb = []
        for ci, (off, sz) in enumerate(CH):
            t = konst.tile([sz, 9, sz], bf16, tag=f"dwd{ci}")
            src = (dwd1 if ci == 0 else dwd2).rearrange("s p m -> p s m")
            nc.gpsimd.dma_start(out=t, in_=src)
            dwd_sb.append(t)

        id128 = konst.tile([128, 128], f32, tag="id128")
        make_identity(nc, id128)
        zcol = konst.tile([128, 16, 1], bf16, tag="zcol")
        nc.gpsimd.memset(zcol, 0.0)
        zrow = konst.tile([128, 1, 132], bf16, tag="zrow")
        nc.gpsimd.memset(zrow, 0.0)

        # ---------- phase 1: G = x x^T via transposed loads ----------
        smctx = ExitStack()
        smp = smctx.enter_context(tc.tile_pool(name="smp", bufs=2, space="PSUM"))
        G_sb = []
        with tc.tile_pool(name="gx", bufs=1) as gxp:
            # one whole-tensor xbar-transpose load; the resulting spatial
            # grouping (stride-128 subsets per partition) is irrelevant for G.
            xT = gxp.tile([128, 128, C], bf16, tag="xT")
            nc.sync.dma_start(out=xT[:], in_=xh[:, :], transpose=True)
            for ci, (off, sz) in enumerate(CH):
                ps = smp.tile([sz, C], f32, tag="smps", bufs=3)
                for k in range(128):
                    nc.tensor.matmul(ps, lhsT=xT[:, k, off:off + sz],
                                     rhs=xT[:, k, :],
                                     start=(k == 0), stop=(k == 127))
                g = konst.tile([sz, C], f32, tag=f"G{ci}")
                nc.scalar.activation(out=g, in_=ps, func=Act.Copy)
                G_sb.append(g)

        big = ctx.enter_context(tc.tile_pool(name="big", bufs=1))

        # ---------- phase 2: attention smalls ----------
        def mm_small(lhsT_tiles, rhs_tiles, mslices, tag, dt=f32):
            """out[m, n] accumulated over the 2 K-chunks; returns psum tiles."""
            outs = []
            for mi, (moff, msz) in enumerate(mslices):
                ps = smp.tile([msz, C], f32, tag="smps", bufs=3)
                for kc in range(2):
                    nc.tensor.matmul(
                        ps, lhsT=lhsT_tiles[kc][:, moff:moff + msz],
                        rhs=rhs_tiles[kc], start=(kc == 0), stop=(kc == 1))
                outs.append(ps)
            return outs

        def to_sbuf(ps_tiles, tag, dt=f32):
            outs = []
            for ci, ps in enumerate(ps_tiles):
                t = st.tile([ps.shape[0], C], dt, tag=f"{tag}{ci}", bufs=1)
                nc.scalar.activation(out=t, in_=ps, func=Act.Copy)
                outs.append(t)
            return outs

        # T_c = G @ Wq^T ; T_b = G @ Wk^T   (lhsT = G, symmetric)
        Tc = to_sbuf(mm_small(G_sb, wqT_sb, CH, "tc"), "Tc")
        Tb = to_sbuf(mm_small(G_sb, wkT_sb, CH, "tb"), "Tb")

        # gram[c,d] = Wq G Wk^T : lhsT = Tc (=G Wq^T, so Tc^T rows=j), rhs=wkT
        gram_sb = to_sbuf(mm_small(Tc, wkT_sb, CH, "gram"), "gram")
        # qq[c,c'] = Wq G Wq^T : lhsT = wqT, rhs = Tc
        qq_ps = mm_small(wqT_sb, Tc, CH, "qq")
        kk_ps = mm_small(wkT_sb, Tb, CH, "kk")

        # ssq via masked row-reduce against identity
        ssq_q, ssq_k = [], []
        for ci, (off, sz) in enumerate(CH):
            scr = st.tile([sz, C], f32, tag=f"scr{ci}", bufs=1)
            scr2 = st.tile([sz, C], f32, tag=f"scr2{ci}", bufs=1)
            sq = st.tile([sz, 1], f32, tag=f"ssqq{ci}", bufs=1)
            nc.vector.tensor_mul(scr, qq_ps[ci], ident_sb[ci])
            nc.vector.reduce_sum(out=sq, in_=scr, axis=mybir.AxisListType.X)
            sk = st.tile([sz, 1], f32, tag=f"ssqk{ci}", bufs=1)
            nc.vector.tensor_mul(scr2, kk_ps[ci], ident_sb[ci])
            nc.vector.reduce_sum(out=sk, in_=scr2, axis=mybir.AxisListType.X)
            ssq_q.append(sq)
            ssq_k.append(sk)

        # scale_q[c] = temp[c] / max(sqrt(ssq_q), eps); rk = 1/max(sqrt(ssq_k),eps)
        scale_q, rk_col = [], []
        for ci, (off, sz) in enumerate(CH):
            a = st.tile([sz, 1], f32, tag=f"sq{ci}", bufs=1)
            nc.scalar.activation(out=a, in_=ssq_q[ci], func=Act.Sqrt)
            nc.vector.tensor_scalar_max(a, a, EPS)
            nc.vector.reciprocal(out=a, in_=a)
            nc.vector.tensor_mul(a, a, temp_sb[ci])
            scale_q.append(a)
            b = st.tile([sz, 1], f32, tag=f"rk{ci}", bufs=1)
            nc.scalar.activation(out=b, in_=ssq_k[ci], func=Act.Sqrt)
            nc.vector.tensor_scalar_max(b, b, EPS)
            nc.vector.reciprocal(out=b, in_=b)
            rk_col.append(b)

        # rk as a broadcast row: transpose [C,1] -> [1,C] on PE, bounce via DRAM
        rk_ps = smp.tile([1, C], f32, tag="smps", bufs=3)
        for ci, (off, sz) in enumerate(CH):
            nc.tensor.matmul(rk_ps[:, off:off + sz], lhsT=rk_col[ci],
                             rhs=id128[0:sz, 0:sz], is_transpose=True,
                             start=True, stop=True,
                             skip_group_check=True)
        rk_row = st.tile([1, C], f32, tag="rkrow", bufs=1)
        nc.scalar.activation(out=rk_row, in_=rk_ps, func=Act.Copy)
        rk_dram = drp.tile([1, C], f32, tag="rkdram")
        nc.sync.dma_start(out=rk_dram, in_=rk_row)
        rk_bc = []
        for ci, (off, sz) in enumerate(CH):
            t = st.tile([sz, C], f32, tag=f"rkbc{ci}", bufs=1)
            src = bass.AP(tensor=rk_dram.tensor, offset=rk_dram.offset,
                          ap=[[0, sz]] + list(rk_dram.ap[1:]))
            nc.gpsimd.dma_start(out=t, in_=src)
            rk_bc.append(t)

        # E = exp(scale_q[c] * rk[d] * gram[c,d]); mask; row-sum; recip
        rs = []
        Em = []
        for ci, (off, sz) in enumerate(CH):
            e = st.tile([sz, C], f32, tag=f"E{ci}", bufs=1)
            nc.vector.scalar_tensor_tensor(
                out=e, in0=gram_sb[ci], scalar=scale_q[ci], in1=rk_bc[ci],
                op0=Alu.mult, op1=Alu.mult)
            nc.scalar.activation(out=e, in_=e, func=Act.Exp)
            em = st.tile([sz, C], f32, tag=f"Em{ci}", bufs=1)
            srow = st.tile([sz, 1], f32, tag=f"srow{ci}", bufs=1)
            nc.vector.tensor_mul(em, e, mask_sb[ci])
            nc.vector.reduce_sum(out=srow, in_=em, axis=mybir.AxisListType.X)
            r = st.tile([sz, 1], f32, tag=f"rs{ci}", bufs=1)
            nc.vector.reciprocal(out=r, in_=srow)
            rs.append(r)
            Em.append(em)

        # A^T (block-diag softmax numerator, transposed) in bf16 for the matmul
        AT_sb = []
        for di, (doff, dsz) in enumerate(CH):
            ps = smp.tile([dsz, C], f32, tag="smps", bufs=3)
            for ci, (coff, csz) in enumerate(CH):
                nc.tensor.matmul(ps[:, coff:coff + csz],
                                 lhsT=Em[ci][:, doff:doff + dsz],
                                 rhs=id128[0:csz, 0:csz], is_transpose=True,
                                 start=True, stop=True,
                                 skip_group_check=True)
            at = st.tile([dsz, C], bf16, tag=f"AT{di}", bufs=1)
            nc.scalar.activation(out=at, in_=ps, func=Act.Copy)
            AT_sb.append(at)

        smctx.close()
        psp = ctx.enter_context(tc.tile_pool(name="psp", bufs=1, space="PSUM"))

        # ---------- phase 3: v = Wv x (full, resident, bf16) ----------
        v_sb = [big.tile([sz, HW], bf16, tag=f"v{ci}", name=f"v{ci}")
                for ci, (off, sz) in enumerate(CH)]
        for n in range(NCH):
            cols = slice(512 * n, 512 * (n + 1))
            xs = [st.tile([sz, 512], bf16, tag=f"xs{ci}", bufs=6, name=f"xs{ci}")
                  for ci, (off, sz) in enumerate(CH)]
            for ci, (off, sz) in enumerate(CH):
                nc.sync.dma_start(out=xs[ci], in_=xh[off:off + sz, cols])
            for mi, (moff, msz) in enumerate(CH):
                ps = psp.tile([msz, 512], f32, tag=f"mmps{mi}", bufs=2)
                for kc in range(2):
                    nc.tensor.matmul(ps, lhsT=wvT_sb[kc][:, moff:moff + msz],
                                     rhs=xs[kc], start=(kc == 0), stop=(kc == 1))
                nc.scalar.activation(out=v_sb[mi][:, cols], in_=ps, func=Act.Copy)

        # ---------- phase 4: u = (A v) / s, written padded at 2 parities ----
        # padded tiles: [sz, 10, 132]; A content at col 2 (serves dj=1 taps +
        # all PE-region taps), B content at col 3 (serves dj=0 and dj=2 taps).
        upA = [[big.tile([sz, 10, 132], bf16, tag=f"uA{ci}", bufs=4,
                         name=f"uA{ci}_{t}")
                for t in range(DWT)] for ci, (off, sz) in enumerate(CH)]
        upB = [[big.tile([sz, 10, 132], bf16, tag=f"uB{ci}", bufs=4,
                         name=f"uB{ci}_{t}")
                if t >= PE_DW_TILES else None
                for t in range(DWT)] for ci, (off, sz) in enumerate(CH)]

        def zc(dst, sz):    # zero a [sz, 10, 1] column strip via ACT
            nc.scalar.activation(out=dst, in_=zcol[0:sz, 0:10, :], func=Act.Copy)

        def zr(dst, sz):    # zero a [sz, 1, 132] row strip via ACT
            nc.scalar.activation(out=dst, in_=zrow[0:sz], func=Act.Copy)

        for ci, (off, sz) in enumerate(CH):
            for t in range(DWT):
                if upB[ci][t] is not None:
                    zc(upB[ci][t][:, :, 2:3], sz)
                    zc(upB[ci][t][:, :, 131:132], sz)
                if t < PE_DW_TILES:
                    # PE-region taps read A cols 1 (dj=0) and 130 (dj=2)
                    zc(upA[ci][t][:, :, 1:2], sz)
                    zc(upA[ci][t][:, :, 130:131], sz)
            zr(upA[ci][0][:, 0:1, :], sz)
            zr(upA[ci][DWT - 1][:, 9:10, :], sz)
            zr(upB[ci][DWT - 1][:, 9:10, :], sz)

        def u_write(mi, ps, rows_psum, t, lr, nrows):
            """copy psum rows [rows_psum, rows_psum+nrows) into tile t at
            local row lr (content rows are local 1..8)."""
            src = ps.rearrange("p (r w) -> p r w", w=128)[
                :, rows_psum:rows_psum + nrows, :]
            nc.scalar.activation(
                out=upA[mi][t][:, lr:lr + nrows, 2:130], in_=src,
                func=Act.Copy, scale=rs[mi])
            if upB[mi][t] is not None:
                nc.scalar.activation(
                    out=upB[mi][t][:, lr:lr + nrows, 3:131], in_=src,
                    func=Act.Copy, scale=rs[mi])

        for n in range(NCH):
            cols = slice(512 * n, 512 * (n + 1))
            t, half = n // 2, n % 2
            for mi, (moff, msz) in enumerate(CH):
                ps = psp.tile([msz, 512], f32, tag=f"mmps{mi}", bufs=2)
                for kc in range(2):
                    nc.tensor.matmul(ps, lhsT=AT_sb[kc][:, moff:moff + msz],
                                     rhs=v_sb[kc][:, cols],
                                     start=(kc == 0), stop=(kc == 1))
                u_write(mi, ps, 0, t, 1 + 4 * half, 4)
                if half == 0 and t >= 1:          # first row -> halo of t-1
                    u_write(mi, ps, 0, t - 1, 9, 1)
                if half == 1 and t <= DWT - 2:    # last row -> halo of t+1
                    u_write(mi, ps, 3, t + 1, 0, 1)

        # ---------- phase 5+6: depthwise 3x3 + projection, streamed --------
        # tap s = 3*di + dj reads local rows [di:di+8]; A cols [2:130] (dj=1),
        # B cols [2:130] (dj=0) / [4:132] (dj=2).
        def tap_src(ci, t, di, dj):
            if dj == 1:
                return upA[ci][t][:, di:di + 8, 2:130]
            return upB[ci][t][:, di:di + 8, 2 + dj:130 + dj]

        for t in range(DWT):
            dwo = [st.tile([sz, 8, 128], bf16, tag=f"dwo{ci}", bufs=3, name=f"dwo{ci}")
                   for ci, (off, sz) in enumerate(CH)]
            if t < PE_DW_TILES:
                for ci, (off, sz) in enumerate(CH):
                    for hf in range(2):
                        ps = psp.tile([sz, 512], f32, tag="dwps", bufs=1)
                        for s in range(9):
                            di, dj = s // 3, s % 3
                            src = upA[ci][t][:, di + 4 * hf:di + 4 * hf + 4,
                                             1 + dj:129 + dj]
                            nc.tensor.matmul(ps, lhsT=dwd_sb[ci][:, s, :],
                                             rhs=src, start=(s == 0),
                                             stop=(s == 8))
                        nc.scalar.activation(
                            out=dwo[ci][:, 4 * hf:4 * hf + 4, :], in_=ps,
                            func=Act.Copy)
            else:
                for ci, (off, sz) in enumerate(CH):
                    nc.vector.tensor_scalar_mul(
                        dwo[ci], tap_src(ci, t, 0, 0), dww_sb[ci][:, 0:1])
                    for s in range(1, 9):
                        di, dj = s // 3, s % 3
                        nc.vector.scalar_tensor_tensor(
                            out=dwo[ci], in0=tap_src(ci, t, di, dj),
                            scalar=dww_sb[ci][:, s:s + 1], in1=dwo[ci],
                            op0=Alu.mult, op1=Alu.add)

            for hf in range(2):
                cols = slice(1024 * t + 512 * hf, 1024 * t + 512 * (hf + 1))
                for mi, (moff, msz) in enumerate(CH):
                    ps = psp.tile([msz, 512], f32, tag="yps", bufs=2)
                    for kc in range(2):
                        nc.tensor.matmul(
                            ps, lhsT=wpT_sb[kc][:, moff:moff + msz],
                            rhs=dwo[kc][:, 4 * hf:4 * hf + 4, :],
                            start=(kc == 0), stop=(kc == 1))
                    ys = st.tile([msz, 512], bf16, tag=f"ys{mi}", bufs=3)
                    nc.scalar.activation(out=ys, in_=ps, func=Act.Copy)
                    nc.sync.dma_start(out=y[moff:moff + msz, cols], in_=ys)


def _prep_host(x, w_qkv, w_dw, w_proj, temperature):
    bf = ml_dtypes.bfloat16
    wq, wk, wv = w_qkv[0:C], w_qkv[C:2 * C], w_qkv[2 * C:3 * C]
    base = {
        "wvT": np.ascontiguousarray(wv.T).astype(bf),
        "wqT": np.ascontiguousarray(wq.T).astype(np.float32),
        "wkT": np.ascontiguousarray(wk.T).astype(np.float32),
        "wpT": np.ascontiguousarray(w_proj.T).astype(bf),
        "tempx": np.repeat(temperature.reshape(HEADS), HD).reshape(C, 1)
                   .astype(np.float32),
        "dww": w_dw[:, 0].reshape(C, 9).astype(np.float32),
    }
    mask = np.zeros((C, C), np.float32)
    for h in range(HEADS):
        mask[h * HD:(h + 1) * HD, h * HD:(h + 1) * HD] = 1.0
    base["maskd"] = mask
    base["identd"] = np.eye(C, dtype=np.float32)
    d1 = np.zeros((9, 128, 128), np.float32)
    d2 = np.zeros((9, 64, 64), np.float32)
    wd = w_dw[:, 0].reshape(C, 9)
    for s in range(9):
        d1[s][np.arange(128), np.arange(128)] = wd[0:128, s]
        d2[s][np.arange(64), np.arange(64)] = wd[128:192, s]
    base["dwd1"] = d1.astype(bf)
    base["dwd2"] = d2.astype(bf)

    in_maps = []
    for i in range(B):
        m = dict(base)
        m["xh"] = np.ascontiguousarray(x[i].reshape(C, HW)).astype(bf)
        in_maps.append(m)
    return in_maps


def _get_runner():
    """Build the jitted 8-core SPMD executor once and cache it; a fresh
    jax.jit per call would re-lower the whole module every time."""
    if "runner" in _CACHE:
        return _CACHE["runner"]
    import jax
    from jax.experimental.shard_map import shard_map
    from jax.sharding import Mesh, PartitionSpec
    import concourse.mybir as mybir
    from concourse import bass2jax

    nc = _CACHE.get("nc")
    if nc is None:
        nc = _CACHE["nc"] = _build_bass()
    bass2jax.install_neuronx_cc_hook()

    partition_name = (nc.partition_id_tensor.name
                      if nc.partition_id_tensor else None)
    in_names, out_names, out_avals, zero_shapes = [], [], [], []
    for alloc in nc.m.functions[0].allocations:
        if not isinstance(alloc, mybir.MemoryLocationSet):
            continue
        name = alloc.memorylocations[0].name
        if alloc.kind == "ExternalInput":
            if name != partition_name:
                in_names.append(name)
        elif alloc.kind == "ExternalOutput":
            shape = tuple(alloc.tensor_shape)
            dtype = mybir.dt.np(alloc.dtype)
            out_names.append(name)
            out_avals.append(jax.core.ShapedArray(shape, dtype))
            zero_shapes.append((shape, dtype))
    n_params = len(in_names)
    all_names = in_names + out_names
    if partition_name is not None:
        all_names = all_names + [partition_name]
    donate = tuple(range(n_params, n_params + len(out_names)))

    def _body(*args):
        operands = list(args)
        if partition_name is not None:
            operands.append(bass2jax.partition_id_tensor())
        outs = bass2jax._bass_exec_p.bind(
            *operands,
            out_avals=tuple(out_avals),
            in_names=tuple(all_names),
            out_names=tuple(out_names),
            lowering_input_output_aliases=(),
            sim_require_finite=True,
            sim_require_nnan=True,
            nc=nc,
        )
        return tuple(outs)

    devices = jax.devices()[:B]
    mesh = Mesh(np.asarray(devices), ("core",))
    specs = (PartitionSpec("core"),) * (n_params + len(out_names))
    fn = jax.jit(
        shard_map(_body, mesh=mesh, in_specs=specs,
                  out_specs=(PartitionSpec("core"),) * len(out_names),
                  check_rep=False),
        donate_argnums=donate, keep_unused=True)

    import jax.numpy as jnp
    from jax.sharding import NamedSharding

    def _mk_zeros():
        return tuple(jnp.zeros((B * s[0], *s[1:]), dt) for (s, dt) in zero_shapes)

    zfn = jax.jit(_mk_zeros,
                  out_shardings=tuple(NamedSharding(mesh, PartitionSpec("core"))
                                      for _ in zero_shapes))
    _CACHE["zeros_fn"] = zfn

    _CACHE["runner"] = (fn, in_names, out_names, out_avals, zero_shapes, n_params)
    return _CACHE["runner"]


def measure_device_ns(in_maps=None, iters=12):
    """Per-run device-exec estimate: pre-upload inputs, queue `iters`
    executions asynchronously (PJRT serializes per device), take the
    slope (t_iters - t_1)/(iters-1).  Dispatch pipelining makes this a
    good proxy for on-device exec time."""
    import jax, time
    fn, in_names, out_names, out_avals, zero_shapes, n_params = _get_runner()
    if in_maps is None:
        in_maps = _CACHE["last_in_maps"]
    concat_in = [
        np.concatenate([in_maps[c][name] for c in range(B)], axis=0)
        for name in in_names
    ]
    din = [jax.device_put(a) for a in concat_in]
    jax.block_until_ready(din)

    def run(k):
        best = None
        for _ in range(3):
            zs = [_CACHE["zeros_fn"]() for _ in range(k)]
            jax.block_until_ready(zs)
            t0 = time.perf_counter()
            outs = [fn(*din, *zs[i]) for i in range(k)]
            jax.block_until_ready(outs)
            dt = time.perf_counter() - t0
            best = dt if best is None else min(best, dt)
        return best

    run(1)  # warm/compile the committed-layout variant
    t1, tk = run(1), run(iters)
    return max(0.0, (tk - t1) / (iters - 1)) * 1e9


def kernel(x, w_qkv, w_dw, w_proj, temperature):
    x = np.asarray(x, np.float32)
    w_qkv = np.asarray(w_qkv, np.float32)
    w_dw = np.asarray(w_dw, np.float32)
    w_proj = np.asarray(w_proj, np.float32)
    temperature = np.asarray(temperature, np.float32)

    fn, in_names, out_names, out_avals, zero_shapes, n_params = _get_runner()
    in_maps = _prep_host(x, w_qkv, w_dw, w_proj, temperature)
    _CACHE["last_in_maps"] = in_maps

    concat_in = [
        np.concatenate([in_maps[c][name] for c in range(B)], axis=0)
        for name in in_names
    ]
    concat_zeros = _CACHE["zeros_fn"]()
    out_arrs = fn(*concat_in, *concat_zeros)
    y = np.asarray(out_arrs[0]).astype(np.float32).reshape(B, C, H, W)
    return y


# revision 30
# speedup vs baseline: 1125.2604x; 1.1541x over previous
"""Channel-attention block (QKV 1x1 -> L2-normalized channel attention ->
depthwise 3x3 -> 1x1 proj) on 8 Trainium2 NeuronCores, data-parallel over
the batch (B=8, C=192, H=W=128, HEADS=16, HD=12).

Key algebraic restructuring: q and k are never materialized on-chip.
With G = x @ x^T ([C,C] Gram over spatial dim), the attention logits are
  logits = Wq G Wk^T   (scaled by 1/(||q_c|| ||k_d||) * temperature)
and the norms are the diagonals of Wq G Wq^T / Wk G Wk^T.  Only v = Wv x
is computed at full spatial width.  The depthwise 3x3 conv runs as
per-channel FMAs on the Vector engine (bf16 2x mode via two padded copies
of the attention output at both column parities) with the first rows
offloaded to the Tensor engine as diagonal-matrix matmuls.
"""

import numpy as np
import ml_dtypes

B, C, H, W = 8, 192, 128, 128
HW = H * W
HEADS = 16
HD = C // HEADS
EPS = 1e-12

NCH = 32          # number of 512-column chunks of HW
DWT = 16          # dw row-tiles (8 rows each)
PE_DW_TILES = 3   # dw tiles 0..PE_DW_TILES-1 computed on PE, rest on DVE
CH = [(0, 128), (128, 64)]   # channel chunks (offset, size)

_CACHE = {}


def _build_bass():
    import concourse.bass as bass
    import concourse.mybir as mybir
    import concourse.tile as tile
    from concourse.masks import make_identity

    f32 = mybir.dt.float32
    bf16 = mybir.dt.bfloat16

    nc = bass.Bass()

    xh = nc.declare_dram_parameter("xh", [C, HW], bf16, isOutput=False)
    wvT = nc.declare_dram_parameter("wvT", [C, C], bf16, isOutput=False)
    wqT = nc.declare_dram_parameter("wqT", [C, C], f32, isOutput=False)
    wkT = nc.declare_dram_parameter("wkT", [C, C], f32, isOutput=False)
    wpT = nc.declare_dram_parameter("wpT", [C, C], bf16, isOutput=False)
    maskd = nc.declare_dram_parameter("maskd", [C, C], f32, isOutput=False)
    identd = nc.declare_dram_parameter("identd", [C, C], f32, isOutput=False)
    tempx = nc.declare_dram_parameter("tempx", [C, 1], f32, isOutput=False)
    dww = nc.declare_dram_parameter("dww", [C, 9], f32, isOutput=False)
    dwd1 = nc.declare_dram_parameter("dwd1", [9, 128, 128], bf16, isOutput=False)
    dwd2 = nc.declare_dram_parameter("dwd2", [9, 64, 64], bf16, isOutput=False)
    y = nc.declare_dram_parameter("y", [C, HW], bf16, isOutput=True)

    with tile.TileContext(nc) as tc:
        _emit(tc, nc, bass, mybir, make_identity, f32, bf16,
              xh, wvT, wqT, wkT, wpT, maskd, identd, tempx, dww, dwd1, dwd2, y)

    patched = _spill_excess_waits(nc.to_json_bytes())
    nc.to_json_bytes = lambda: patched
    return nc


def _spill_excess_waits(bir_json: bytes) -> bytes:
    """walrus allows ~1 sync-wait per lowered ISA struct; Tile can attach
    several to one instruction.  Move every wait beyond the first onto an
    injected same-engine NoOp placed immediately before the instruction."""
    import json as _json

    j = _json.loads(bir_json)
    n = [0]
    for fn in j["functions"]:
        for blk in fn["blocks"]:
            out = []
            for inst in blk["instructions"]:
                si = inst.get("sync_info")
                keep = 0 if inst.get("opcode") == "ISA" else 1
                if (si and si.get("on_wait") and len(si["on_wait"]) > keep
                        and inst.get("opcode") != "EventSemaphore"):
                    waits = si["on_wait"]
                    for w in (waits[:-1] if keep else waits):
                        n[0] += 1
                        out.append({
                            "debug": inst.get("debug", 0),
                            "engine": inst["engine"],
                            "ins": [], "outs": [],
                            "name": f"WSPILL-{n[0]}",
                            "opcode": "NoOp",
                            "sync_info": {"on_update": [], "on_wait": [w]},
                        })
                    si["on_wait"] = [waits[-1]] if keep else []
                out.append(inst)
            blk["instructions"] = out
    return _json.dumps(j).encode()


def _emit(tc, nc, bass, mybir, make_identity, f32, bf16,
          xh, wvT, wqT, wkT, wpT, maskd, identd, tempx, dww, dwd1, dwd2, y):
    from contextlib import ExitStack

    Alu = mybir.AluOpType
    Act = mybir.ActivationFunctionType

    ctx = ExitStack()
    with ctx:
        konst = ctx.enter_context(tc.tile_pool(name="konst", bufs=1))
        st = ctx.enter_context(tc.tile_pool(name="st", bufs=2))
        drp = ctx.enter_context(tc.tile_pool(name="drp", bufs=1, space="DRAM"))

        # ---------- constant loads ----------
        def load_pair(src, dt, name):
            ts = []
            for ci, (off, sz) in enumerate(CH):
                t = konst.tile([sz, C], dt, tag=f"{name}{ci}")
                nc.gpsimd.dma_start(out=t, in_=src[off:off + sz, :])
                ts.append(t)
            return ts

        wvT_sb = load_pair(wvT, bf16, "wvT")
        wqT_sb = load_pair(wqT, f32, "wqT")
        wkT_sb = load_pair(wkT, f32, "wkT")
        wpT_sb = load_pair(wpT, bf16, "wpT")
        mask_sb = load_pair(maskd, f32, "mask")
        ident_sb = load_pair(identd, f32, "ident")

        temp_sb, dww_sb = [], []
        for ci, (off, sz) in enumerate(CH):
            t = konst.tile([sz, 1], f32, tag=f"temp{ci}")
            nc.gpsimd.dma_start(out=t, in_=tempx[off:off + sz, :])
            temp_sb.append(t)
            d = konst.tile([sz, 9], f32, tag=f"dww{ci}")
            nc.gpsimd.dma_start(out=d, in_=dww[off:off + sz, :])
            dww_sb.append(d)

        dwd_sb = []
        for ci, (off, sz) in enumerate(CH):
            t = konst.tile([sz, 9, sz], bf16, tag=f"dwd{ci}")
            src = (dwd1 if ci == 0 else dwd2).rearrange("s p m -> p s m")
            nc.gpsimd.dma_start(out=t, in_=src)
            dwd_sb.append(t)

        id128 = konst.tile([128, 128], f32, tag="id128")
        make_identity(nc, id128)
        zcol = konst.tile([128, 16, 1], bf16, tag="zcol")
        nc.gpsimd.memset(zcol, 0.0)
        zrow = konst.tile([128, 1, 132], bf16, tag="zrow")
        nc.gpsimd.memset(zrow, 0.0)

        # ---------- phase 1: G = x x^T via transposed loads ----------
        smctx = ExitStack()
        smp = smctx.enter_context(tc.tile_pool(name="smp", bufs=2, space="PSUM"))
        G_sb = []
        with tc.tile_pool(name="gx", bufs=1) as gxp:
            # one whole-tensor xbar-transpose load; the resulting spatial
            # grouping (stride-128 subsets per partition) is irrelevant for G.
            xT = gxp.tile([128, 128, C], bf16, tag="xT")
            nc.sync.dma_start(out=xT[:], in_=xh[:, :], transpose=True)
            for ci, (off, sz) in enumerate(CH):
                ps = smp.tile([sz, C], f32, tag="smps", bufs=3)
                for k in range(128):
                    nc.tensor.matmul(ps, lhsT=xT[:, k, off:off + sz],
                                     rhs=xT[:, k, :],
                                     start=(k == 0), stop=(k == 127))
                g = konst.tile([sz, C], f32, tag=f"G{ci}")
                nc.scalar.activation(out=g, in_=ps, func=Act.Copy)
                G_sb.append(g)

        big = ctx.enter_context(tc.tile_pool(name="big", bufs=1))

        # ---------- phase 2: attention smalls ----------
        def mm_small(lhsT_tiles, rhs_tiles, mslices, tag, dt=f32):
            """out[m, n] accumulated over the 2 K-chunks; returns psum tiles."""
            outs = []
            for mi, (moff, msz) in enumerate(mslices):
                ps = smp.tile([msz, C], f32, tag="smps", bufs=3)
                for kc in range(2):
                    nc.tensor.matmul(
                        ps, lhsT=lhsT_tiles[kc][:, moff:moff + msz],
                        rhs=rhs_tiles[kc], start=(kc == 0), stop=(kc == 1))
                outs.append(ps)
            return outs

        def to_sbuf(ps_tiles, tag, dt=f32):
            outs = []
            for ci, ps in enumerate(ps_tiles):
                t = st.tile([ps.shape[0], C], dt, tag=f"{tag}{ci}", bufs=1)
                nc.scalar.activation(out=t, in_=ps, func=Act.Copy)
                outs.append(t)
            return outs

        # T_c = G @ Wq^T ; T_b = G @ Wk^T   (lhsT = G, symmetric)
        Tc = to_sbuf(mm_small(G_sb, wqT_sb, CH, "tc"), "Tc")
        Tb = to_sbuf(mm_small(G_sb, wkT_sb, CH, "tb"), "Tb")

        # gram[c,d] = Wq G Wk^T : lhsT = Tc (=G Wq^T, so Tc^T rows=j), rhs=wkT
        gram_sb = to_sbuf(mm_small(Tc, wkT_sb, CH, "gram"), "gram")
        # qq[c,c'] = Wq G Wq^T : lhsT = wqT, rhs = Tc
        qq_ps = mm_small(wqT_sb, Tc, CH, "qq")
        kk_ps = mm_small(wkT_sb, Tb, CH, "kk")

        # ssq via masked row-reduce against identity
        ssq_q, ssq_k = [], []
        for ci, (off, sz) in enumerate(CH):
            scr = st.tile([sz, C], f32, tag=f"scr{ci}", bufs=1)
            scr2 = st.tile([sz, C], f32, tag=f"scr2{ci}", bufs=1)
            sq = st.tile([sz, 1], f32, tag=f"ssqq{ci}", bufs=1)
            nc.vector.tensor_mul(scr, qq_ps[ci], ident_sb[ci])
            nc.vector.reduce_sum(out=sq, in_=scr, axis=mybir.AxisListType.X)
            sk = st.tile([sz, 1], f32, tag=f"ssqk{ci}", bufs=1)
            nc.vector.tensor_mul(scr2, kk_ps[ci], ident_sb[ci])
            nc.vector.reduce_sum(out=sk, in_=scr2, axis=mybir.AxisListType.X)
            ssq_q.append(sq)
            ssq_k.append(sk)

        # scale_q[c] = temp[c] / max(sqrt(ssq_q), eps); rk = 1/max(sqrt(ssq_k),eps)
        scale_q, rk_col = [], []
        for ci, (off, sz) in enumerate(CH):
            a = st.tile([sz, 1], f32, tag=f"sq{ci}", bufs=1)
            nc.scalar.activation(out=a, in_=ssq_q[ci], func=Act.Sqrt)
            nc.vector.tensor_scalar_max(a, a, EPS)
            nc.vector.reciprocal(out=a, in_=a)
            nc.vector.tensor_mul(a, a, temp_sb[ci])
            scale_q.append(a)
            b = st.tile([sz, 1], f32, tag=f"rk{ci}", bufs=1)
            nc.scalar.activation(out=b, in_=ssq_k[ci], func=Act.Sqrt)
            nc.vector.tensor_scalar_max(b, b, EPS)
            nc.vector.reciprocal(out=b, in_=b)
            rk_col.append(b)

        # rk as a broadcast row: transpose [C,1] -> [1,C] on PE, bounce via DRAM
        rk_ps = smp.tile([1, C], f32, tag="smps", bufs=3)
        for ci, (off, sz) in enumerate(CH):
            nc.tensor.matmul(rk_ps[:, off:off + sz], lhsT=rk_col[ci],
                             rhs=id128[0:sz, 0:sz], is_transpose=True,
                             start=True, stop=True,
                             skip_group_check=True)
        rk_row = st.tile([1, C], f32, tag="rkrow", bufs=1)
        nc.scalar.activation(out=rk_row, in_=rk_ps, func=Act.Copy)
        rk_dram = drp.tile([1, C], f32, tag="rkdram")
        nc.sync.dma_start(out=rk_dram, in_=rk_row)
        rk_bc = []
        for ci, (off, sz) in enumerate(CH):
            t = st.tile([sz, C], f32, tag=f"rkbc{ci}", bufs=1)
            src = bass.AP(tensor=rk_dram.tensor, offset=rk_dram.offset,
                          ap=[[0, sz]] + list(rk_dram.ap[1:]))
            nc.gpsimd.dma_start(out=t, in_=src)
            rk_bc.append(t)

        # E = exp(scale_q[c] * rk[d] * gram[c,d]); mask; row-sum; recip
        rs = []
        Em = []
        for ci, (off, sz) in enumerate(CH):
            e = st.tile([sz, C], f32, tag=f"E{ci}", bufs=1)
            nc.vector.scalar_tensor_tensor(
                out=e, in0=gram_sb[ci], scalar=scale_q[ci], in1=rk_bc[ci],
                op0=Alu.mult, op1=Alu.mult)
            nc.scalar.activation(out=e, in_=e, func=Act.Exp)
            em = st.tile([sz, C], f32, tag=f"Em{ci}", bufs=1)
            srow = st.tile([sz, 1], f32, tag=f"srow{ci}", bufs=1)
            nc.vector.tensor_mul(em, e, mask_sb[ci])
            nc.vector.reduce_sum(out=srow, in_=em, axis=mybir.AxisListType.X)
            r = st.tile([sz, 1], f32, tag=f"rs{ci}", bufs=1)
            nc.vector.reciprocal(out=r, in_=srow)
            rs.append(r)
            Em.append(em)

        # A^T (block-diag softmax numerator, transposed) in bf16 for the matmul
        AT_sb = []
        for di, (doff, dsz) in enumerate(CH):
            ps = smp.tile([dsz, C], f32, tag="smps", bufs=3)
            for ci, (coff, csz) in enumerate(CH):
                nc.tensor.matmul(ps[:, coff:coff + csz],
                                 lhsT=Em[ci][:, doff:doff + dsz],
                                 rhs=id128[0:csz, 0:csz], is_transpose=True,
                                 start=True, stop=True,
                                 skip_group_check=True)
            at = st.tile([dsz, C], bf16, tag=f"AT{di}", bufs=1)
            nc.scalar.activation(out=at, in_=ps, func=Act.Copy)
            AT_sb.append(at)

        smctx.close()
        psp = ctx.enter_context(tc.tile_pool(name="psp", bufs=1, space="PSUM"))

        # ---------- phase 3: v = Wv x (full, resident, bf16) ----------
        v_sb = [big.tile([sz, HW], bf16, tag=f"v{ci}", name=f"v{ci}")
                for ci, (off, sz) in enumerate(CH)]
        for n in range(NCH):
            cols = slice(512 * n, 512 * (n + 1))
            xs = [st.tile([sz, 512], bf16, tag=f"xs{ci}", bufs=6, name=f"xs{ci}")
                  for ci, (off, sz) in enumerate(CH)]
            for ci, (off, sz) in enumerate(CH):
                nc.sync.dma_start(out=xs[ci], in_=xh[off:off + sz, cols])
            for mi, (moff, msz) in enumerate(CH):
                ps = psp.tile([msz, 512], f32, tag=f"mmps{mi}", bufs=2)
                for kc in range(2):
                    nc.tensor.matmul(ps, lhsT=wvT_sb[kc][:, moff:moff + msz],
                                     rhs=xs[kc], start=(kc == 0), stop=(kc == 1))
                nc.scalar.activation(out=v_sb[mi][:, cols], in_=ps, func=Act.Copy)

        # ---------- phase 4: u = (A v) / s, written padded at 2 parities ----
        # padded tiles: [sz, 10, 132]; A content at col 2 (serves dj=1 taps +
        # all PE-region taps), B content at col 3 (serves dj=0 and dj=2 taps).
        upA = [[big.tile([sz, 10, 132], bf16, tag=f"uA{ci}", bufs=4,
                         name=f"uA{ci}_{t}")
                for t in range(DWT)] for ci, (off, sz) in enumerate(CH)]
        upB = [[big.tile([sz, 10, 132], bf16, tag=f"uB{ci}", bufs=4,
                         name=f"uB{ci}_{t}")
                if t >= PE_DW_TILES else None
                for t in range(DWT)] for ci, (off, sz) in enumerate(CH)]

        def zc(dst, sz):    # zero a [sz, 10, 1] column strip via ACT
            nc.scalar.activation(out=dst, in_=zcol[0:sz, 0:10, :], func=Act.Copy)

        def zr(dst, sz):    # zero a [sz, 1, 132] row strip via ACT
            nc.scalar.activation(out=dst, in_=zrow[0:sz], func=Act.Copy)

        for ci, (off, sz) in enumerate(CH):
            for t in range(DWT):
                if upB[ci][t] is not None:
                    zc(upB[ci][t][:, :, 2:3], sz)
                    zc(upB[ci][t][:, :, 131:132], sz)
                if t < PE_DW_TILES:
                    # PE-region taps read A cols 1 (dj=0) and 130 (dj=2)
                    zc(upA[ci][t][:, :, 1:2], sz)
                    zc(upA[ci][t][:, :, 130:131], sz)
            zr(upA[ci][0][:, 0:1, :], sz)
            zr(upA[ci][DWT - 1][:, 9:10, :], sz)
            zr(upB[ci][DWT - 1][:, 9:10, :], sz)

        def u_write(mi, ps, rows_psum, t, lr, nrows):
            """copy psum rows [rows_psum, rows_psum+nrows) into tile t at
            local row lr (content rows are local 1..8)."""
            src = ps.rearrange("p (r w) -> p r w", w=128)[
                :, rows_psum:rows_psum + nrows, :]
            nc.scalar.activation(
                out=upA[mi][t][:, lr:lr + nrows, 2:130], in_=src,
                func=Act.Copy, scale=rs[mi])
            if upB[mi][t] is not None:
                nc.scalar.activation(
                    out=upB[mi][t][:, lr:lr + nrows, 3:131], in_=src,
                    func=Act.Copy, scale=rs[mi])

        for n in range(NCH):
            cols = slice(512 * n, 512 * (n + 1))
            t, half = n // 2, n % 2
            for mi, (moff, msz) in enumerate(CH):
                ps = psp.tile([msz, 512], f32, tag=f"mmps{mi}", bufs=2)
                for kc in range(2):
                    nc.tensor.matmul(ps, lhsT=AT_sb[kc][:, moff:moff + msz],
                                     rhs=v_sb[kc][:, cols],
                                     start=(kc == 0), stop=(kc == 1))
                u_write(mi, ps, 0, t, 1 + 4 * half, 4)
                if half == 0 and t >= 1:          # first row -> halo of t-1
                    u_write(mi, ps, 0, t - 1, 9, 1)
                if half == 1 and t <= DWT - 2:    # last row -> halo of t+1
                    u_write(mi, ps, 3, t + 1, 0, 1)

        # ---------- phase 5+6: depthwise 3x3 + projection, streamed --------
        # tap s = 3*di + dj reads local rows [di:di+8]; A cols [2:130] (dj=1),
        # B cols [2:130] (dj=0) / [4:132] (dj=2).
        def tap_src(ci, t, di, dj):
            if dj == 1:
                return upA[ci][t][:, di:di + 8, 2:130]
            return upB[ci][t][:, di:di + 8, 2 + dj:130 + dj]

        for t in range(DWT):
            dwo = [st.tile([sz, 8, 128], bf16, tag=f"dwo{ci}", bufs=3, name=f"dwo{ci}")
                   for ci, (off, sz) in enumerate(CH)]
            if t < PE_DW_TILES:
                for ci, (off, sz) in enumerate(CH):
                    for hf in range(2):
                        ps = psp.tile([sz, 512], f32, tag="dwps", bufs=1)
                        for s in range(9):
                            di, dj = s // 3, s % 3
                            src = upA[ci][t][:, di + 4 * hf:di + 4 * hf + 4,
                                             1 + dj:129 + dj]
                            nc.tensor.matmul(ps, lhsT=dwd_sb[ci][:, s, :],
                                             rhs=src, start=(s == 0),
                                             stop=(s == 8))
                        nc.scalar.activation(
                            out=dwo[ci][:, 4 * hf:4 * hf + 4, :], in_=ps,
                            func=Act.Copy)
            else:
                for ci, (off, sz) in enumerate(CH):
                    nc.vector.tensor_scalar_mul(
                        dwo[ci], tap_src(ci, t, 0, 0), dww_sb[ci][:, 0:1])
                    for s in range(1, 9):
                        di, dj = s // 3, s % 3
                        nc.vector.scalar_tensor_tensor(
                            out=dwo[ci], in0=tap_src(ci, t, di, dj),
                            scalar=dww_sb[ci][:, s:s + 1], in1=dwo[ci],
                            op0=Alu.mult, op1=Alu.add)

            for hf in range(2):
                cols = slice(1024 * t + 512 * hf, 1024 * t + 512 * (hf + 1))
                for mi, (moff, msz) in enumerate(CH):
                    ps = psp.tile([msz, 512], f32, tag="yps", bufs=2)
                    for kc in range(2):
                        nc.tensor.matmul(
                            ps, lhsT=wpT_sb[kc][:, moff:moff + msz],
                            rhs=dwo[kc][:, 4 * hf:4 * hf + 4, :],
                            start=(kc == 0), stop=(kc == 1))
                    ys = st.tile([msz, 512], bf16, tag=f"ys{mi}", bufs=3)
                    nc.scalar.activation(out=ys, in_=ps, func=Act.Copy)
                    nc.sync.dma_start(out=y[moff:moff + msz, cols], in_=ys)


def _prep_host(x, w_qkv, w_dw, w_proj, temperature):
    bf = ml_dtypes.bfloat16
    wq, wk, wv = w_qkv[0:C], w_qkv[C:2 * C], w_qkv[2 * C:3 * C]
    base = {
        "wvT": np.ascontiguousarray(wv.T).astype(bf),
        "wqT": np.ascontiguousarray(wq.T).astype(np.float32),
        "wkT": np.ascontiguousarray(wk.T).astype(np.float32),
        "wpT": np.ascontiguousarray(w_proj.T).astype(bf),
        "tempx": np.repeat(temperature.reshape(HEADS), HD).reshape(C, 1)
                   .astype(np.float32),
        "dww": w_dw[:, 0].reshape(C, 9).astype(np.float32),
    }
    mask = np.zeros((C, C), np.float32)
    for h in range(HEADS):
        mask[h * HD:(h + 1) * HD, h * HD:(h + 1) * HD] = 1.0
    base["maskd"] = mask
    base["identd"] = np.eye(C, dtype=np.float32)
    d1 = np.zeros((9, 128, 128), np.float32)
    d2 = np.zeros((9, 64, 64), np.float32)
    wd = w_dw[:, 0].reshape(C, 9)
    for s in range(9):
        d1[s][np.arange(128), np.arange(128)] = wd[0:128, s]
        d2[s][np.arange(64), np.arange(64)] = wd[128:192, s]
    base["dwd1"] = d1.astype(bf)
    base["dwd2"] = d2.astype(bf)

    xcat = x.reshape(B * C, HW).astype(bf)   # one-pass cast of the batch
    in_maps = []
    for i in range(B):
        m = dict(base)
        m["xh"] = xcat[i * C:(i + 1) * C]
        in_maps.append(m)
    return in_maps


def _get_runner():
    """Build the jitted 8-core SPMD executor once and cache it; a fresh
    jax.jit per call would re-lower the whole module every time."""
    if "runner" in _CACHE:
        return _CACHE["runner"]
    import jax
    from jax.experimental.shard_map import shard_map
    from jax.sharding import Mesh, PartitionSpec
    import concourse.mybir as mybir
    from concourse import bass2jax

    nc = _CACHE.get("nc")
    if nc is None:
        nc = _CACHE["nc"] = _build_bass()
    bass2jax.install_neuronx_cc_hook()

    partition_name = (nc.partition_id_tensor.name
                      if nc.partition_id_tensor else None)
    in_names, out_names, out_avals, zero_shapes = [], [], [], []
    for alloc in nc.m.functions[0].allocations:
        if not isinstance(alloc, mybir.MemoryLocationSet):
            continue
        name = alloc.memorylocations[0].name
        if alloc.kind == "ExternalInput":
            if name != partition_name:
                in_names.append(name)
        elif alloc.kind == "ExternalOutput":
            shape = tuple(alloc.tensor_shape)
            dtype = mybir.dt.np(alloc.dtype)
            out_names.append(name)
            out_avals.append(jax.core.ShapedArray(shape, dtype))
            zero_shapes.append((shape, dtype))
    n_params = len(in_names)
    all_names = in_names + out_names
    if partition_name is not None:
        all_names = all_names + [partition_name]
    donate = tuple(range(n_params, n_params + len(out_names)))

    def _body(*args):
        operands = list(args)
        if partition_name is not None:
            operands.append(bass2jax.partition_id_tensor())
        outs = bass2jax._bass_exec_p.bind(
            *operands,
            out_avals=tuple(out_avals),
            in_names=tuple(all_names),
            out_names=tuple(out_names),
            lowering_input_output_aliases=(),
            sim_require_finite=True,
            sim_require_nnan=True,
            nc=nc,
        )
        return tuple(outs)

    devices = jax.devices()[:B]
    mesh = Mesh(np.asarray(devices), ("core",))
    specs = (PartitionSpec("core"),) * (n_params + len(out_names))
    fn = jax.jit(
        shard_map(_body, mesh=mesh, in_specs=specs,
                  out_specs=(PartitionSpec("core"),) * len(out_names),
                  check_rep=False),
        donate_argnums=donate, keep_unused=True)

    import jax.numpy as jnp
    from jax.sharding import NamedSharding

    def _mk_zeros():
        return tuple(jnp.zeros((B * s[0], *s[1:]), dt) for (s, dt) in zero_shapes)

    zfn = jax.jit(_mk_zeros,
                  out_shardings=tuple(NamedSharding(mesh, PartitionSpec("core"))
                                      for _ in zero_shapes))
    _CACHE["zeros_fn"] = zfn

    _CACHE["runner"] = (fn, in_names, out_names, out_avals, zero_shapes, n_params)
    return _CACHE["runner"]


def measure_device_ns(in_maps=None, iters=12):
    """Per-run device-exec estimate: pre-upload inputs, queue `iters`
    executions asynchronously (PJRT serializes per device), take the
    slope (t_iters - t_1)/(iters-1).  Dispatch pipelining makes this a
    good proxy for on-device exec time."""
    import jax, time
    fn, in_names, out_names, out_avals, zero_shapes, n_params = _get_runner()
    if in_maps is None:
        in_maps = _CACHE["last_in_maps"]
    concat_in = [
        np.concatenate([in_maps[c][name] for c in range(B)], axis=0)
        for name in in_names
    ]
    din = [jax.device_put(a) for a in concat_in]
    jax.block_until_ready(din)

    def run(k):
        best = None
        for _ in range(3):
            zs = [_CACHE["zeros_fn"]() for _ in range(k)]
            jax.block_until_ready(zs)
            t0 = time.perf_counter()
            outs = [fn(*din, *zs[i]) for i in range(k)]
            jax.block_until_ready(outs)
            dt = time.perf_counter() - t0
            best = dt if best is None else min(best, dt)
        return best

    run(1)  # warm/compile the committed-layout variant
    t1, tk = run(1), run(iters)
    return max(0.0, (tk - t1) / (iters - 1)) * 1e9


def kernel(x, w_qkv, w_dw, w_proj, temperature):
    x = np.asarray(x, np.float32)
    w_qkv = np.asarray(w_qkv, np.float32)
    w_dw = np.asarray(w_dw, np.float32)
    w_proj = np.asarray(w_proj, np.float32)
    temperature = np.asarray(temperature, np.float32)

    in_maps = _prep_host(x, w_qkv, w_dw, w_proj, temperature)
    _CACHE["last_in_maps"] = in_maps

    from concourse._compat import axon_active
    if not axon_active():
        # native /dev/neuron* path: direct NRT execution
        from concourse.bass_utils import run_bass_kernel_spmd
        if "nc" not in _CACHE:
            _CACHE["nc"] = _build_bass()
        res = run_bass_kernel_spmd(_CACHE["nc"], in_maps,
                                   core_ids=list(range(B)))
        _CACHE["last_results"] = res
        out = np.empty((B, C, H, W), np.float32)
        for i in range(B):
            out[i] = res.results[i]["y"].astype(np.float32).reshape(C, H, W)
        return out

    fn, in_names, out_names, out_avals, zero_shapes, n_params = _get_runner()
    concat_in = [
        np.concatenate([in_maps[c][name] for c in range(B)], axis=0)
        for name in in_names
    ]
    concat_zeros = _CACHE["zeros_fn"]()
    out_arrs = fn(*concat_in, *concat_zeros)
    y = np.asarray(out_arrs[0]).astype(np.float32).reshape(B, C, H, W)
    return y
